# revision 24
# baseline (speedup 1.0000x reference)
"""MLA prefill kernel (fp16) for Trainium2, 8 NeuronCores.

Sharding: data-parallel over batch (2) x tensor-parallel over heads
(16 heads -> 4 per core).  Core c handles batch c//4, head group c%4.
Each core computes its full attention block plus a partial output
projection; the host sums the 4 per-group partials per batch.

All matmul operands are fp16 (1 cycle/row on the PE; fp32/f32r ran in
fp32_mode=HIGH at ~2.5 cycles/row; fp16's 10 mantissa bits keep the
rel-err ~8x below bf16), accumulation stays f32 in PSUM.
Everything is computed transposed ([feature, L]) so matmul lhsT/rhs
operands are produced directly, except V (L-major for the PV matmul),
which stays resident in SBUF.  Scores are computed transposed
(S^T = K Q^T, [Lk, Lq]) so softmax's sum runs through the PV matmul
via an appended ones-column; exp needs no max-subtraction (scores are
O(10)).  RoPE pair mixing runs along partitions via a +-1 pair-swap
matmul (J) plus two elementwise multiplies and an add.

The attention inner loop is software-pipelined two key-tiles ahead
(PV(t) trails QK(t+2)) so the tensor engine does not stall on the exp
activation; causal masks run on the Pool engine.  PSUM: 4 banks hold
the two heads' PV accumulators, 4 banks rotate for scores/projections.
The softmax denominator is inverted as exp(-ln(d)) on the Act engine
and broadcast across partitions with a K=1 ones matmul; each chunk's
normalization tail and output projection are deferred past the next
chunk's QKV so that latency hides behind tensor-engine work.  w_out is
SBUF-resident with the per-head rope halves packed in pairs so the
output projection runs 6 full-K=128 matmuls.
"""

import math
import os
import sys

sys.path.insert(0, "/opt/trn_rl_repo")

import numpy as np

import concourse.bass as bass
import concourse.mybir as mybir
import concourse.tile as tile
from concourse.bass import ds
from concourse.bass_utils import run_bass_kernel_spmd

H, DH, RK, RD = 16, 128, 512, 64
B, L, E = 2, 2048, 2048
HPG = 4                      # heads per core
NCORE = 8
DV = DH + RD                 # 192
SCALE = 1.0 / math.sqrt(DV)
CH = 512                     # Lq chunk
NCH = L // CH                # 4
LT = L // 128                # 16 key tiles
ET = E // 128                # 16
VROW = HPG * (DV + 1)        # 772: per-head 192 v dims + ones col

F32 = mybir.dt.float32
FP16 = mybir.dt.float16
AF = mybir.ActivationFunctionType
NPFP16 = np.float16

_CACHE = {}


def _split_excess_waits(nc, limit=1):
    """walrus on this toolchain accepts at most one sem-wait per
    instruction; hoist extras onto same-engine no-ops just before."""
    f = nc.m.functions[0]
    for bb in f.blocks:
        new_list = []
        changed = False
        for inst in bb.instructions:
            si = inst.sync_info
            if si is not None and si.on_wait is not None and len(si.on_wait) > limit:
                waits = list(si.on_wait)
                changed = True
                n = 0
                while len(waits) > limit:
                    chunk, waits = waits[:limit], waits[limit:]
                    new_list.append(mybir.InstNoOp(
                        name=f"{inst.name}-ws{n}",
                        sync_info=mybir.SyncInfo(on_wait=chunk, on_update=[]),
                        bass_nofuse=True,
                        engine=inst.engine,
                    ))
                    n += 1
                inst.sync_info = mybir.SyncInfo(on_wait=waits, on_update=si.on_update)
            new_list.append(inst)
        if changed:
            bb.instructions[:] = new_list
    return nc


def _build():
    nc = bass.Bass(target_bir_lowering=False, trn_type="TRN2")

    xt = nc.dram_tensor("xt", [NCH, 128, ET, CH], FP16, kind="ExternalInput")
    w1 = nc.dram_tensor("w1", [11, 128, ET, 128], FP16, kind="ExternalInput")
    wuk = nc.dram_tensor("wuk", [RK, HPG * DH], FP16, kind="ExternalInput")
    wuv = nc.dram_tensor("wuv", [RK, HPG * DV], FP16, kind="ExternalInput")
    wo = nc.dram_tensor("wo", [128, 6, E], FP16, kind="ExternalInput")
    cost = nc.dram_tensor("cost", [128, L], FP16, kind="ExternalInput")
    sint = nc.dram_tensor("sint", [128, L], FP16, kind="ExternalInput")
    jt = nc.dram_tensor("jt", [128, 128], FP16, kind="ExternalInput")
    triu = nc.dram_tensor("triu", [128, 128], FP16, kind="ExternalInput")
    ones1 = nc.dram_tensor("ones1", [1, 128], FP16, kind="ExternalInput")
    outt = nc.dram_tensor("outt", [L, E], FP16, kind="ExternalOutput")

    from contextlib import ExitStack

    with tile.TileContext(nc) as tc:
        with ExitStack() as ctx:
            ctx.enter_context(nc.allow_low_precision(
                reason="bf16 kernel; all contractions accumulate in f32 psum"))
            pool_specs = [
                ("consts", 1, None), ("res", 1, None),
                ("xt_p", 2, None), ("w1_p", 4, None),
                ("qt_p", 2, None), ("rq_p", 2, None), ("ckv_p", 2, None),
                ("pt_p", 6, None), ("tmp_p", 2, None),
                ("oz_p", 2, None), ("fin_p", 3, None),
                ("acc_p", 2, "PSUM"), ("rot_p", 4, "PSUM"),
            ]
            pools = {}
            for pname, pbufs, pspace in pool_specs:
                kw = {"name": pname, "bufs": pbufs}
                if pspace:
                    kw["space"] = pspace
                pools[pname] = ctx.enter_context(tc.tile_pool(**kw))
            (consts, res, xt_p, w1_p, qt_p, rq_p, ckv_p, pt_p, tmp_p,
             oz_p, fin_p, acc_p, rot_p) = (pools[s[0]] for s in pool_specs)

            def rot():
                return rot_p.tile([128, 512], F32, tag="ps", name="ps")

            # ---- constants / resident weights.  Only the small consts and
            # cos/sin go ahead of chunk 0's xt/w1 input DMAs; the bulky
            # resident weights (wuk/wuv/wo) are dispatched after chunk 0's
            # QKV emission so the first matmul isn't stuck behind them.
            jt_t = consts.tile([128, 128], FP16, tag="jt", name="jt")
            nc.sync.dma_start(out=jt_t[:], in_=jt.ap())
            tri_t = consts.tile([128, 128], FP16, tag="tri", name="tri")
            nc.sync.dma_start(out=tri_t[:], in_=triu.ap())
            one_t = consts.tile([1, 128], FP16, tag="one", name="one")
            nc.sync.dma_start(out=one_t[:], in_=ones1.ap())
            cos_sb = res.tile([128, L], FP16, tag="cos", name="cos")
            sin_sb = res.tile([128, L], FP16, tag="sin", name="sin")
            wukt = res.tile([128, RK // 128, HPG * DH], FP16, tag="wukt", name="wukt")
            wuvt = res.tile([128, RK // 128, HPG * DV], FP16, tag="wuvt", name="wuvt")
            wo_t = res.tile([128, 6, E], FP16, tag="wo", name="wo")

            def load_residents():
                nc.scalar.dma_start(out=cos_sb[:], in_=cost.ap())
                nc.scalar.dma_start(out=sin_sb[:], in_=sint.ap())
                nc.sync.dma_start(
                    out=wukt[:], in_=wuk.ap().rearrange("(t p) n -> p t n", p=128))
                nc.sync.dma_start(
                    out=wuvt[:], in_=wuv.ap().rearrange("(t p) n -> p t n", p=128))
                nc.scalar.dma_start(out=wo_t[:], in_=wo.ap())

            ktc = res.tile([128, HPG, L], FP16, tag="ktc", name="ktc")   # K content, transposed
            rkd = res.tile([128, L], FP16, tag="rkd", name="rkd")        # roped k_rope, dup rows
            vd = res.tile([128, LT, VROW], FP16, tag="vd", name="vd")    # V resident (L-major + ones)
            vdv = vd[:].rearrange("p t (h x) -> p t h x", x=DV + 1)
            nc.gpsimd.memset(vdv[:, :, :, DV], 1.0)                      # ones columns

            # d-tiles of the fused QKV projection: (kind, idx)
            dtiles = ([("q", i) for i in range(HPG)]
                      + [("ckv", i) for i in range(RK // 128)]
                      + [("rq", i) for i in range(2)]
                      + [("rk", 0)])

            # deferred-work closures (prev chunk's norm tail + output proj),
            # emitted after the next chunk's QKV so the slow reciprocal and
            # the oz writes hide behind tensor-engine work
            pending = []

            for c in range(NCH):
                ccols = ds(c * CH, CH)

                # ================= QKV(c): [1344, CH] = W1^T @ x^T =======
                xtt = xt_p.tile([128, ET, CH], FP16, tag="xtt", name="xtt")
                nc.sync.dma_start(out=xtt[:], in_=xt.ap()[c])
                qtc = qt_p.tile([128, HPG, CH], FP16, tag="qtc", name="qtc")
                rq = rq_p.tile([128, 2, CH], FP16, tag="rq", name="rq")
                ckv = ckv_p.tile([128, RK // 128, CH], FP16, tag="ckv", name="ckv")

                for di, (kind, idx) in enumerate(dtiles):
                    w1s = w1_p.tile([128, ET, 128], FP16, tag="w1s", name="w1s")
                    nc.sync.dma_start(out=w1s[:], in_=w1.ap()[di])
                    dw = RD if kind == "rk" else 128
                    ps = rot()
                    for e in range(ET):
                        nc.tensor.matmul(ps[:dw, :CH], w1s[:, e, :dw], xtt[:, e, :],
                                         start=(e == 0), stop=(e == ET - 1))
                    if kind == "q":
                        nc.scalar.copy(out=qtc[:, idx, :], in_=ps[:, :CH])
                    elif kind == "ckv":
                        nc.vector.tensor_copy(ckv[:, idx, :], ps[:, :CH])
                    elif kind == "rq":
                        nc.vector.tensor_copy(rq[:, idx, :], ps[:, :CH])
                    else:  # pre-rope k_rope at partitions 0:64
                        nc.vector.tensor_copy(rkd[0:RD, ccols], ps[:RD, :CH])

                if c == 0:
                    load_residents()
                for fn in pending:
                    fn()
                pending = []

                # ================= RoPE(c) ===============================
                # roped = R * cos + (J @ R) * sin   (pairs along partitions)
                for i in range(2):  # q_rope, two head-pair tiles
                    swp = rot()
                    nc.tensor.matmul(swp[:, :CH], jt_t[:, :], rq[:, i, :],
                                     start=True, stop=True)
                    t1 = tmp_p.tile([128, CH], FP16, tag="ropet", name="ropet")
                    nc.vector.tensor_mul(t1[:], rq[:, i, :], cos_sb[:, ccols])
                    nc.vector.tensor_mul(rq[:, i, :], swp[:, :CH], sin_sb[:, ccols])
                    nc.vector.tensor_add(rq[:, i, :], rq[:, i, :], t1[:])
                swp = rot()
                nc.tensor.matmul(swp[:RD, :CH], jt_t[:RD, :RD], rkd[0:RD, ccols],
                                 start=True, stop=True)
                t1 = tmp_p.tile([128, CH], FP16, tag="ropet", name="ropet")
                nc.vector.tensor_mul(t1[:RD, :], rkd[0:RD, ccols], cos_sb[0:RD, ccols])
                nc.vector.tensor_mul(rkd[0:RD, ccols], swp[:RD, :CH], sin_sb[0:RD, ccols])
                nc.vector.tensor_add(rkd[0:RD, ccols], rkd[0:RD, ccols], t1[:RD, :])
                # duplicate roped k_rope to partitions 64:128 (for odd heads)
                nc.sync.dma_start(out=rkd[RD:128, ccols], in_=rkd[0:RD, ccols])

                # ================= UP-K(c): K^T = Wuk^T @ c_kv^T =========
                for h in range(HPG):
                    ps = rot()
                    for kt in range(RK // 128):
                        nc.tensor.matmul(ps[:, :CH], wukt[:, kt, ds(128 * h, 128)],
                                         ckv[:, kt, :],
                                         start=(kt == 0), stop=(kt == RK // 128 - 1))
                    nc.scalar.copy(out=ktc[:, h, ccols], in_=ps[:, :CH])

                # ================= UP-V(c): V = c_kv @ Wuv (L-major) =====
                for lti in range(4):
                    lt = 4 * c + lti
                    for nb in range(2):
                        psv = rot()
                        for kt in range(RK // 128):
                            nc.tensor.matmul(psv[:, :384],
                                             ckv[:, kt, ds(128 * lti, 128)],
                                             wuvt[:, kt, ds(384 * nb, 384)],
                                             start=(kt == 0), stop=(kt == RK // 128 - 1))
                        for q in range(2):
                            hh = 2 * nb + q
                            nc.vector.tensor_copy(vd[:, lt, ds((DV + 1) * hh, DV)],
                                                  psv[:, ds(DV * q, DV)])

                # ================= ATT(c): head pairs, 1-tile pipelined ==
                ntk = 4 * c + 4
                oz = oz_p.tile([128, 6, CH], FP16, tag="oz", name="oz")

                def attn_half(hp, inject=None):
                    heads = (2 * hp, 2 * hp + 1)
                    A = [(acc_p.tile([128, 512], F32, tag="acc1", name="acc1"),
                          acc_p.tile([128, 512], F32, tag="acc2", name="acc2"))
                         for _ in range(2)]
                    geom = []
                    for t in range(ntk):
                        j = t - 4 * c
                        off = 128 * j if j >= 0 else 0
                        geom.append((off, CH - off, j >= 0))
                    sps_l = {}
                    pt_l = {}

                    def emit_qk(t):
                        off, n, _ = geom[t]
                        sps_l[t] = []
                        for q in range(2):
                            h = heads[q]
                            hb = RD * (h % 2)
                            sps = rot()
                            nc.tensor.matmul(sps[:, ds(off, n)],
                                             ktc[:, h, ds(128 * t, 128)],
                                             qtc[:, h, ds(off, n)],
                                             start=True, stop=False)
                            nc.tensor.matmul(sps[:, ds(off, n)],
                                             rkd[hb:hb + RD, ds(128 * t, 128)],
                                             rq[hb:hb + RD, h // 2, ds(off, n)],
                                             start=False, stop=True)
                            sps_l[t].append(sps)

                    def emit_exp(t):
                        off, n, diag = geom[t]
                        pt_l[t] = []
                        for q in range(2):
                            pt = pt_p.tile([128, CH], FP16, tag="pt", name="pt")
                            nc.scalar.activation(pt[:, ds(off, n)],
                                                 sps_l[t][q][:, ds(off, n)],
                                                 AF.Exp, scale=SCALE)
                            if diag:
                                # on Pool: keeps the mask off the DVE, whose
                                # queue carries the slow reciprocals
                                nc.gpsimd.tensor_mul(pt[:, ds(off, 128)],
                                                     pt[:, ds(off, 128)], tri_t[:])
                            pt_l[t].append(pt)

                    def emit_pv(t):
                        off, n, _ = geom[t]
                        for q in range(2):
                            h = heads[q]
                            ps1, ps2 = A[q]
                            pt = pt_l[t][q]
                            nc.tensor.matmul(ps1[:, ds(off, n)],
                                             vd[:, t, ds((DV + 1) * h, 128)],
                                             pt[:, ds(off, n)],
                                             start=(t == 0), stop=(t == ntk - 1),
                                             skip_group_check=True)
                            nc.tensor.matmul(ps2[:DV - DH + 1, ds(off, n)],
                                             vd[:, t, ds((DV + 1) * h + DH, DV - DH + 1)],
                                             pt[:, ds(off, n)],
                                             start=(t == 0), stop=(t == ntk - 1),
                                             skip_group_check=True)

                    # 2-tile software pipeline: PV(t) trails QK(t+2) so the
                    # exp (and diag mask) latency is fully covered by PE work
                    emit_qk(0)
                    emit_qk(1)
                    emit_exp(0)
                    for t in range(2, ntk):
                        emit_qk(t)
                        emit_pv(t - 2)
                        emit_exp(t - 1)
                        if inject is not None and t == min(3, ntk - 1):
                            inject()
                            inject = None
                    emit_pv(ntk - 2)
                    emit_exp(ntk - 1)
                    emit_pv(ntk - 1)
                    if inject is not None:
                        inject()

                    # softmax denominator reciprocal as exp(-ln(d)) on the
                    # Act engine: ~3x faster than the DVE InstReciprocal and
                    # keeps the DVE free for evacuations
                    rrs = []
                    for q in range(2):
                        lnd = tmp_p.tile([1, CH], F32, tag="lnd", name="lnd",
                                         bufs=4)
                        nc.scalar.activation(lnd[:], A[q][1][RD:RD + 1, :CH],
                                             AF.Ln)
                        rr16 = tmp_p.tile([1, CH], FP16, tag="rr16", name="rr16",
                                          bufs=4)
                        nc.scalar.activation(rr16[:], lnd[:], AF.Exp, scale=-1.0)
                        rrs.append(rr16)

                    def finish_norm():
                        for q in range(2):
                            h = heads[q]
                            ps1, ps2 = A[q]
                            rb = rot()
                            nc.tensor.matmul(rb[:, :CH], one_t[:, :], rrs[q][:, :],
                                             start=True, stop=True)
                            rbs = tmp_p.tile([128, CH], FP16, tag="rbs", name="rbs")
                            nc.scalar.copy(out=rbs[:], in_=rb[:, :CH])
                            nc.vector.tensor_mul(oz[:, h, :], ps1[:, :CH], rbs[:])
                            hb = RD * (h % 2)
                            nc.vector.tensor_mul(oz[hb:hb + RD, 4 + hp, :],
                                                 ps2[0:RD, :CH], rbs[0:RD, :])
                    return finish_norm

                fin0 = attn_half(0)
                fin1 = attn_half(1, inject=fin0)
                pending.append(fin1)

                # ===== FINAL(c): out = attn @ WO, deferred past QKV(c+1) =
                def make_final(c, oz):
                    def emit_final():
                        for eg in range(E // CH):
                            for ls in range(CH // 128):
                                fps = rot()
                                for kt in range(6):
                                    nc.tensor.matmul(fps[:, :CH],
                                                     oz[:, kt, ds(128 * ls, 128)],
                                                     wo_t[:, kt, ds(CH * eg, CH)],
                                                     start=(kt == 0), stop=(kt == 5))
                                fin = fin_p.tile([128, CH], FP16, tag="fin",
                                                 name="fin")
                                nc.scalar.copy(out=fin[:], in_=fps[:, :CH])
                                nc.gpsimd.dma_start(
                                    out=outt.ap()[ds(c * CH + 128 * ls, 128),
                                                  ds(CH * eg, CH)],
                                    in_=fin[:])
                    return emit_final

                pending.append(make_final(c, oz))

            for fn in pending:
                fn()
            pending = []

    _split_excess_waits(nc)
    return nc


def _prep_inputs(x, cos_table, sin_table, wq, wkv_down, w_up, w_out):
    f32 = np.float32
    wq3 = np.asarray(wq, f32).reshape(E, H, DV)
    wup3 = np.asarray(w_up, f32).reshape(RK, H, 2 * DH + RD)
    wo3 = np.asarray(w_out, f32).reshape(H, DV, E)
    wkv = np.asarray(wkv_down, f32)

    cosI = np.repeat(np.asarray(cos_table, f32)[:L], 2, axis=1).T  # [64, L]
    sinI = np.repeat(np.asarray(sin_table, f32)[:L], 2, axis=1).T
    cost = np.ascontiguousarray(np.concatenate([cosI, cosI], 0)).astype(NPFP16)
    sint = np.ascontiguousarray(np.concatenate([sinI, sinI], 0)).astype(NPFP16)
    J = np.zeros((128, 128), f32)
    for i in range(64):
        J[2 * i, 2 * i + 1] = -1.0
        J[2 * i + 1, 2 * i] = 1.0
    jt = np.ascontiguousarray(J.T).astype(NPFP16)
    triu = np.ascontiguousarray(np.triu(np.ones((128, 128), f32))).astype(NPFP16)

    in_maps = []
    for core in range(NCORE):
        b, g = core // HPG, core % HPG
        hs = slice(HPG * g, HPG * g + HPG)
        xT = np.asarray(x, f32)[b].T                       # [E, L]
        xt_pack = np.ascontiguousarray(
            xT.reshape(ET, 128, NCH, CH).transpose(2, 1, 0, 3)).astype(NPFP16)
        wq_c = wq3[:, hs, :DH].reshape(E, HPG * DH)
        wq_r = wq3[:, hs, DH:].reshape(E, HPG * RD)
        w1_flat = np.concatenate([wq_c, wkv[:, :RK], wq_r, wkv[:, RK:]], axis=1)
        # pack into 11 d-strips [128, ET, 128] (last strip: 64 cols, zero-pad)
        w1_pack = np.zeros((11, 128, ET, 128), f32)
        offs = [128 * i for i in range(10)] + [1280]
        wids = [128] * 10 + [64]
        for di, (o, w) in enumerate(zip(offs, wids)):
            w1_pack[di, :, :, :w] = (
                w1_flat[:, o:o + w].reshape(ET, 128, w).transpose(1, 0, 2))
        # w_out resident: 4 content strips + 2 rope-pair strips
        wo_pack = np.zeros((128, 6, E), f32)
        wog = wo3[hs]                                      # [4, 192, E]
        for kt in range(HPG):
            wo_pack[:, kt, :] = wog[kt, :DH, :]
        for hp in range(2):
            wo_pack[0:RD, 4 + hp, :] = wog[2 * hp, DH:, :]
            wo_pack[RD:128, 4 + hp, :] = wog[2 * hp + 1, DH:, :]
        in_maps.append({
            "xt": xt_pack,
            "w1": w1_pack.astype(NPFP16),
            "wuk": np.ascontiguousarray(
                wup3[:, hs, :DH].reshape(RK, HPG * DH)).astype(NPFP16),
            "wuv": np.ascontiguousarray(
                wup3[:, hs, DH:].reshape(RK, HPG * DV)).astype(NPFP16),
            "wo": wo_pack.astype(NPFP16),
            "cost": cost,
            "sint": sint,
            "jt": jt,
            "triu": triu,
            "ones1": np.ones((1, 128), NPFP16),
        })
    return in_maps


def kernel(x, cos_table, sin_table, wq, wkv_down, w_up, w_out, _want_perf=False):
    if "nc" not in _CACHE:
        _CACHE["nc"] = _build()
    nc = _CACHE["nc"]
    in_maps = _prep_inputs(x, cos_table, sin_table, wq, wkv_down, w_up, w_out)
    res = run_bass_kernel_spmd(nc, in_maps, core_ids=list(range(NCORE)),
                               trace=bool(_want_perf),
                               tmpdir=os.environ.get("BASS_TMPDIR") or None)
    out = np.zeros((B, L, E), np.float32)
    for core in range(NCORE):
        b = core // HPG
        out[b] += res.results[core]["outt"].astype(np.float32)
    if _want_perf:
        return out, res
    return out


# revision 25
# speedup vs baseline: 1.0077x; 1.0077x over previous
"""MLA prefill kernel (fp16) for Trainium2, 8 NeuronCores.

Sharding: data-parallel over batch (2) x tensor-parallel over heads
(16 heads -> 4 per core).  Core c handles batch c//4, head group c%4.
Each core computes its full attention block plus a partial output
projection; the host sums the 4 per-group partials per batch.

All matmul operands are fp16 (1 cycle/row on the PE; fp32/f32r ran in
fp32_mode=HIGH at ~2.5 cycles/row; fp16's 10 mantissa bits keep the
rel-err ~8x below bf16), accumulation stays f32 in PSUM.
Everything is computed transposed ([feature, L]) so matmul lhsT/rhs
operands are produced directly, except V (L-major for the PV matmul),
which stays resident in SBUF.  Scores are computed transposed
(S^T = K Q^T, [Lk, Lq]) so softmax's sum runs through the PV matmul
via an appended ones-column; exp needs no max-subtraction (scores are
O(10)).  RoPE pair mixing runs along partitions via a +-1 pair-swap
matmul (J) plus two elementwise multiplies and an add.

The attention inner loop is software-pipelined two key-tiles ahead
(PV(t) trails QK(t+2)) so the tensor engine does not stall on the exp
activation; causal masks run on the Pool engine.  PSUM: 4 banks hold
the two heads' PV accumulators, 4 banks rotate for scores/projections.
The softmax denominator is inverted as exp(-ln(d)) on the Act engine
and broadcast across partitions with a K=1 ones matmul; each chunk's
normalization tail and output projection are deferred past the next
chunk's QKV so that latency hides behind tensor-engine work.  w_out is
SBUF-resident with the per-head rope halves packed in pairs so the
output projection runs 6 full-K=128 matmuls.
"""

import math
import os
import sys

sys.path.insert(0, "/opt/trn_rl_repo")

import numpy as np

import concourse.bass as bass
import concourse.mybir as mybir
import concourse.tile as tile
from concourse.bass import ds
from concourse.bass_utils import run_bass_kernel_spmd

H, DH, RK, RD = 16, 128, 512, 64
B, L, E = 2, 2048, 2048
HPG = 4                      # heads per core
NCORE = 8
DV = DH + RD                 # 192
SCALE = 1.0 / math.sqrt(DV)
CH = 512                     # Lq chunk
NCH = L // CH                # 4
LT = L // 128                # 16 key tiles
ET = E // 128                # 16
VROW = HPG * (DV + 1)        # 772: per-head 192 v dims + ones col

F32 = mybir.dt.float32
FP16 = mybir.dt.float16
AF = mybir.ActivationFunctionType
NPFP16 = np.float16

_CACHE = {}


def _split_excess_waits(nc, limit=1):
    """walrus on this toolchain accepts at most one sem-wait per
    instruction; hoist extras onto same-engine no-ops just before."""
    f = nc.m.functions[0]
    for bb in f.blocks:
        new_list = []
        changed = False
        for inst in bb.instructions:
            si = inst.sync_info
            if si is not None and si.on_wait is not None and len(si.on_wait) > limit:
                waits = list(si.on_wait)
                changed = True
                n = 0
                while len(waits) > limit:
                    chunk, waits = waits[:limit], waits[limit:]
                    new_list.append(mybir.InstNoOp(
                        name=f"{inst.name}-ws{n}",
                        sync_info=mybir.SyncInfo(on_wait=chunk, on_update=[]),
                        bass_nofuse=True,
                        engine=inst.engine,
                    ))
                    n += 1
                inst.sync_info = mybir.SyncInfo(on_wait=waits, on_update=si.on_update)
            new_list.append(inst)
        if changed:
            bb.instructions[:] = new_list
    return nc


def _build():
    nc = bass.Bass(target_bir_lowering=False, trn_type="TRN2")

    xt = nc.dram_tensor("xt", [NCH, 128, ET, CH], FP16, kind="ExternalInput")
    w1 = nc.dram_tensor("w1", [11, 128, ET, 128], FP16, kind="ExternalInput")
    wuk = nc.dram_tensor("wuk", [RK, HPG * DH], FP16, kind="ExternalInput")
    wuv = nc.dram_tensor("wuv", [RK, HPG * DV], FP16, kind="ExternalInput")
    wo = nc.dram_tensor("wo", [128, 6, E], FP16, kind="ExternalInput")
    cost = nc.dram_tensor("cost", [128, L], FP16, kind="ExternalInput")
    sint = nc.dram_tensor("sint", [128, L], FP16, kind="ExternalInput")
    jt = nc.dram_tensor("jt", [128, 128], FP16, kind="ExternalInput")
    triu = nc.dram_tensor("triu", [128, 128], FP16, kind="ExternalInput")
    ones1 = nc.dram_tensor("ones1", [1, 128], FP16, kind="ExternalInput")
    outt = nc.dram_tensor("outt", [L, E], FP16, kind="ExternalOutput")

    from contextlib import ExitStack

    with tile.TileContext(nc) as tc:
        with ExitStack() as ctx:
            ctx.enter_context(nc.allow_low_precision(
                reason="bf16 kernel; all contractions accumulate in f32 psum"))
            pool_specs = [
                ("consts", 1, None), ("res", 1, None),
                ("xt_p", 2, None), ("w1_p", 4, None),
                ("qt_p", 2, None), ("rq_p", 2, None), ("ckv_p", 2, None),
                ("pt_p", 6, None), ("tmp_p", 2, None),
                ("oz_p", 2, None), ("fin_p", 3, None),
                ("acc_p", 2, "PSUM"), ("rot_p", 4, "PSUM"),
            ]
            pools = {}
            for pname, pbufs, pspace in pool_specs:
                kw = {"name": pname, "bufs": pbufs}
                if pspace:
                    kw["space"] = pspace
                pools[pname] = ctx.enter_context(tc.tile_pool(**kw))
            (consts, res, xt_p, w1_p, qt_p, rq_p, ckv_p, pt_p, tmp_p,
             oz_p, fin_p, acc_p, rot_p) = (pools[s[0]] for s in pool_specs)

            def rot():
                return rot_p.tile([128, 512], F32, tag="ps", name="ps")

            # ---- constants / resident weights.  Only the small consts and
            # cos/sin go ahead of chunk 0's xt/w1 input DMAs; the bulky
            # resident weights (wuk/wuv/wo) are dispatched after chunk 0's
            # QKV emission so the first matmul isn't stuck behind them.
            jt_t = consts.tile([128, 128], FP16, tag="jt", name="jt")
            nc.sync.dma_start(out=jt_t[:], in_=jt.ap())
            tri_t = consts.tile([128, 128], FP16, tag="tri", name="tri")
            nc.sync.dma_start(out=tri_t[:], in_=triu.ap())
            one_t = consts.tile([1, 128], FP16, tag="one", name="one")
            nc.sync.dma_start(out=one_t[:], in_=ones1.ap())
            cos_sb = res.tile([128, L], FP16, tag="cos", name="cos")
            sin_sb = res.tile([128, L], FP16, tag="sin", name="sin")
            wukt = res.tile([128, RK // 128, HPG * DH], FP16, tag="wukt", name="wukt")
            wuvt = res.tile([128, RK // 128, HPG * DV], FP16, tag="wuvt", name="wuvt")
            wo_t = res.tile([128, 6, E], FP16, tag="wo", name="wo")

            def load_residents():
                nc.scalar.dma_start(out=cos_sb[:], in_=cost.ap())
                nc.scalar.dma_start(out=sin_sb[:], in_=sint.ap())
                nc.sync.dma_start(
                    out=wukt[:], in_=wuk.ap().rearrange("(t p) n -> p t n", p=128))
                nc.sync.dma_start(
                    out=wuvt[:], in_=wuv.ap().rearrange("(t p) n -> p t n", p=128))
                nc.scalar.dma_start(out=wo_t[:], in_=wo.ap())

            ktc = res.tile([128, HPG, L], FP16, tag="ktc", name="ktc")   # K content, transposed
            rkd = res.tile([128, L], FP16, tag="rkd", name="rkd")        # roped k_rope, dup rows
            vd = res.tile([128, LT, VROW], FP16, tag="vd", name="vd")    # V resident (L-major + ones)
            vdv = vd[:].rearrange("p t (h x) -> p t h x", x=DV + 1)
            nc.gpsimd.memset(vdv[:, :, :, DV], 1.0)                      # ones columns

            # d-tiles of the fused QKV projection: (kind, idx)
            dtiles = ([("q", i) for i in range(HPG)]
                      + [("ckv", i) for i in range(RK // 128)]
                      + [("rq", i) for i in range(2)]
                      + [("rk", 0)])

            # deferred-work closures (prev chunk's norm tail + output proj),
            # emitted after the next chunk's QKV so the slow reciprocal and
            # the oz writes hide behind tensor-engine work
            pending = []

            for c in range(NCH):
                ccols = ds(c * CH, CH)

                # ================= QKV(c): [1344, CH] = W1^T @ x^T =======
                xtt = xt_p.tile([128, ET, CH], FP16, tag="xtt", name="xtt")
                if c == 0:
                    # sliced load: the first dtile's accumulation starts as
                    # soon as the first quarter lands (DMA fabric ramps
                    # slowly right after kernel start)
                    for s in range(4):
                        nc.sync.dma_start(out=xtt[:, ds(4 * s, 4), :],
                                          in_=xt.ap()[c][:, ds(4 * s, 4), :])
                else:
                    nc.sync.dma_start(out=xtt[:], in_=xt.ap()[c])
                qtc = qt_p.tile([128, HPG, CH], FP16, tag="qtc", name="qtc")
                rq = rq_p.tile([128, 2, CH], FP16, tag="rq", name="rq")
                ckv = ckv_p.tile([128, RK // 128, CH], FP16, tag="ckv", name="ckv")

                for di, (kind, idx) in enumerate(dtiles):
                    w1s = w1_p.tile([128, ET, 128], FP16, tag="w1s", name="w1s")
                    nc.sync.dma_start(out=w1s[:], in_=w1.ap()[di])
                    dw = RD if kind == "rk" else 128
                    ps = rot()
                    for e in range(ET):
                        nc.tensor.matmul(ps[:dw, :CH], w1s[:, e, :dw], xtt[:, e, :],
                                         start=(e == 0), stop=(e == ET - 1))
                    if kind == "q":
                        nc.scalar.copy(out=qtc[:, idx, :], in_=ps[:, :CH])
                    elif kind == "ckv":
                        nc.vector.tensor_copy(ckv[:, idx, :], ps[:, :CH])
                    elif kind == "rq":
                        nc.vector.tensor_copy(rq[:, idx, :], ps[:, :CH])
                    else:  # pre-rope k_rope at partitions 0:64
                        nc.vector.tensor_copy(rkd[0:RD, ccols], ps[:RD, :CH])

                if c == 0:
                    load_residents()
                for fn in pending:
                    fn()
                pending = []

                # ================= RoPE(c) ===============================
                # roped = R * cos + (J @ R) * sin   (pairs along partitions)
                for i in range(2):  # q_rope, two head-pair tiles
                    swp = rot()
                    nc.tensor.matmul(swp[:, :CH], jt_t[:, :], rq[:, i, :],
                                     start=True, stop=True)
                    t1 = tmp_p.tile([128, CH], FP16, tag="ropet", name="ropet")
                    nc.vector.tensor_mul(t1[:], rq[:, i, :], cos_sb[:, ccols])
                    nc.vector.tensor_mul(rq[:, i, :], swp[:, :CH], sin_sb[:, ccols])
                    nc.vector.tensor_add(rq[:, i, :], rq[:, i, :], t1[:])
                swp = rot()
                nc.tensor.matmul(swp[:RD, :CH], jt_t[:RD, :RD], rkd[0:RD, ccols],
                                 start=True, stop=True)
                t1 = tmp_p.tile([128, CH], FP16, tag="ropet", name="ropet")
                nc.vector.tensor_mul(t1[:RD, :], rkd[0:RD, ccols], cos_sb[0:RD, ccols])
                nc.vector.tensor_mul(rkd[0:RD, ccols], swp[:RD, :CH], sin_sb[0:RD, ccols])
                nc.vector.tensor_add(rkd[0:RD, ccols], rkd[0:RD, ccols], t1[:RD, :])
                # duplicate roped k_rope to partitions 64:128 (for odd heads)
                nc.sync.dma_start(out=rkd[RD:128, ccols], in_=rkd[0:RD, ccols])

                # ================= UP-K(c): K^T = Wuk^T @ c_kv^T =========
                for h in range(HPG):
                    ps = rot()
                    for kt in range(RK // 128):
                        nc.tensor.matmul(ps[:, :CH], wukt[:, kt, ds(128 * h, 128)],
                                         ckv[:, kt, :],
                                         start=(kt == 0), stop=(kt == RK // 128 - 1))
                    nc.scalar.copy(out=ktc[:, h, ccols], in_=ps[:, :CH])

                # ================= UP-V(c): V = c_kv @ Wuv (L-major) =====
                for lti in range(4):
                    lt = 4 * c + lti
                    for nb in range(2):
                        psv = rot()
                        for kt in range(RK // 128):
                            nc.tensor.matmul(psv[:, :384],
                                             ckv[:, kt, ds(128 * lti, 128)],
                                             wuvt[:, kt, ds(384 * nb, 384)],
                                             start=(kt == 0), stop=(kt == RK // 128 - 1))
                        for q in range(2):
                            hh = 2 * nb + q
                            nc.vector.tensor_copy(vd[:, lt, ds((DV + 1) * hh, DV)],
                                                  psv[:, ds(DV * q, DV)])

                # ================= ATT(c): head pairs, 1-tile pipelined ==
                ntk = 4 * c + 4
                oz = oz_p.tile([128, 6, CH], FP16, tag="oz", name="oz")

                def attn_half(hp, inject=None):
                    heads = (2 * hp, 2 * hp + 1)
                    A = [(acc_p.tile([128, 512], F32, tag="acc1", name="acc1"),
                          acc_p.tile([128, 512], F32, tag="acc2", name="acc2"))
                         for _ in range(2)]
                    geom = []
                    for t in range(ntk):
                        j = t - 4 * c
                        off = 128 * j if j >= 0 else 0
                        geom.append((off, CH - off, j >= 0))
                    sps_l = {}
                    pt_l = {}

                    def emit_qk(t):
                        off, n, _ = geom[t]
                        sps_l[t] = []
                        for q in range(2):
                            h = heads[q]
                            hb = RD * (h % 2)
                            sps = rot()
                            nc.tensor.matmul(sps[:, ds(off, n)],
                                             ktc[:, h, ds(128 * t, 128)],
                                             qtc[:, h, ds(off, n)],
                                             start=True, stop=False)
                            nc.tensor.matmul(sps[:, ds(off, n)],
                                             rkd[hb:hb + RD, ds(128 * t, 128)],
                                             rq[hb:hb + RD, h // 2, ds(off, n)],
                                             start=False, stop=True)
                            sps_l[t].append(sps)

                    def emit_exp(t):
                        off, n, diag = geom[t]
                        pt_l[t] = []
                        for q in range(2):
                            pt = pt_p.tile([128, CH], FP16, tag="pt", name="pt")
                            nc.scalar.activation(pt[:, ds(off, n)],
                                                 sps_l[t][q][:, ds(off, n)],
                                                 AF.Exp, scale=SCALE)
                            if diag:
                                # on Pool: keeps the mask off the DVE, whose
                                # queue carries the slow reciprocals
                                nc.gpsimd.tensor_mul(pt[:, ds(off, 128)],
                                                     pt[:, ds(off, 128)], tri_t[:])
                            pt_l[t].append(pt)

                    def emit_pv(t):
                        off, n, _ = geom[t]
                        for q in range(2):
                            h = heads[q]
                            ps1, ps2 = A[q]
                            pt = pt_l[t][q]
                            nc.tensor.matmul(ps1[:, ds(off, n)],
                                             vd[:, t, ds((DV + 1) * h, 128)],
                                             pt[:, ds(off, n)],
                                             start=(t == 0), stop=(t == ntk - 1),
                                             skip_group_check=True)
                            nc.tensor.matmul(ps2[:DV - DH + 1, ds(off, n)],
                                             vd[:, t, ds((DV + 1) * h + DH, DV - DH + 1)],
                                             pt[:, ds(off, n)],
                                             start=(t == 0), stop=(t == ntk - 1),
                                             skip_group_check=True)

                    # 2-tile software pipeline: PV(t) trails QK(t+2) so the
                    # exp (and diag mask) latency is fully covered by PE work
                    emit_qk(0)
                    emit_qk(1)
                    emit_exp(0)
                    for t in range(2, ntk):
                        emit_qk(t)
                        emit_pv(t - 2)
                        emit_exp(t - 1)
                        if inject is not None and t == min(3, ntk - 1):
                            inject()
                            inject = None
                    emit_pv(ntk - 2)
                    emit_exp(ntk - 1)
                    emit_pv(ntk - 1)
                    if inject is not None:
                        inject()

                    # softmax denominator reciprocal as exp(-ln(d)) on the
                    # Act engine: ~3x faster than the DVE InstReciprocal and
                    # keeps the DVE free for evacuations
                    rrs = []
                    for q in range(2):
                        lnd = tmp_p.tile([1, CH], F32, tag="lnd", name="lnd",
                                         bufs=4)
                        nc.scalar.activation(lnd[:], A[q][1][RD:RD + 1, :CH],
                                             AF.Ln)
                        rr16 = tmp_p.tile([1, CH], FP16, tag="rr16", name="rr16",
                                          bufs=4)
                        nc.scalar.activation(rr16[:], lnd[:], AF.Exp, scale=-1.0)
                        rrs.append(rr16)

                    def finish_norm():
                        for q in range(2):
                            h = heads[q]
                            ps1, ps2 = A[q]
                            rb = rot()
                            nc.tensor.matmul(rb[:, :CH], one_t[:, :], rrs[q][:, :],
                                             start=True, stop=True)
                            rbs = tmp_p.tile([128, CH], FP16, tag="rbs", name="rbs")
                            nc.scalar.copy(out=rbs[:], in_=rb[:, :CH])
                            nc.vector.tensor_mul(oz[:, h, :], ps1[:, :CH], rbs[:])
                            hb = RD * (h % 2)
                            nc.vector.tensor_mul(oz[hb:hb + RD, 4 + hp, :],
                                                 ps2[0:RD, :CH], rbs[0:RD, :])
                    return finish_norm

                fin0 = attn_half(0)
                fin1 = attn_half(1, inject=fin0)
                pending.append(fin1)

                # ===== FINAL(c): out = attn @ WO, deferred past QKV(c+1) =
                def make_final(c, oz):
                    def emit_final():
                        for eg in range(E // CH):
                            for ls in range(CH // 128):
                                fps = rot()
                                for kt in range(6):
                                    nc.tensor.matmul(fps[:, :CH],
                                                     oz[:, kt, ds(128 * ls, 128)],
                                                     wo_t[:, kt, ds(CH * eg, CH)],
                                                     start=(kt == 0), stop=(kt == 5))
                                fin = fin_p.tile([128, CH], FP16, tag="fin",
                                                 name="fin")
                                nc.scalar.copy(out=fin[:], in_=fps[:, :CH])
                                nc.gpsimd.dma_start(
                                    out=outt.ap()[ds(c * CH + 128 * ls, 128),
                                                  ds(CH * eg, CH)],
                                    in_=fin[:])
                    return emit_final

                pending.append(make_final(c, oz))

            for fn in pending:
                fn()
            pending = []

    _split_excess_waits(nc)
    return nc


def _prep_inputs(x, cos_table, sin_table, wq, wkv_down, w_up, w_out):
    f32 = np.float32
    wq3 = np.asarray(wq, f32).reshape(E, H, DV)
    wup3 = np.asarray(w_up, f32).reshape(RK, H, 2 * DH + RD)
    wo3 = np.asarray(w_out, f32).reshape(H, DV, E)
    wkv = np.asarray(wkv_down, f32)

    cosI = np.repeat(np.asarray(cos_table, f32)[:L], 2, axis=1).T  # [64, L]
    sinI = np.repeat(np.asarray(sin_table, f32)[:L], 2, axis=1).T
    cost = np.ascontiguousarray(np.concatenate([cosI, cosI], 0)).astype(NPFP16)
    sint = np.ascontiguousarray(np.concatenate([sinI, sinI], 0)).astype(NPFP16)
    J = np.zeros((128, 128), f32)
    for i in range(64):
        J[2 * i, 2 * i + 1] = -1.0
        J[2 * i + 1, 2 * i] = 1.0
    jt = np.ascontiguousarray(J.T).astype(NPFP16)
    triu = np.ascontiguousarray(np.triu(np.ones((128, 128), f32))).astype(NPFP16)

    in_maps = []
    for core in range(NCORE):
        b, g = core // HPG, core % HPG
        hs = slice(HPG * g, HPG * g + HPG)
        xT = np.asarray(x, f32)[b].T                       # [E, L]
        xt_pack = np.ascontiguousarray(
            xT.reshape(ET, 128, NCH, CH).transpose(2, 1, 0, 3)).astype(NPFP16)
        wq_c = wq3[:, hs, :DH].reshape(E, HPG * DH)
        wq_r = wq3[:, hs, DH:].reshape(E, HPG * RD)
        w1_flat = np.concatenate([wq_c, wkv[:, :RK], wq_r, wkv[:, RK:]], axis=1)
        # pack into 11 d-strips [128, ET, 128] (last strip: 64 cols, zero-pad)
        w1_pack = np.zeros((11, 128, ET, 128), f32)
        offs = [128 * i for i in range(10)] + [1280]
        wids = [128] * 10 + [64]
        for di, (o, w) in enumerate(zip(offs, wids)):
            w1_pack[di, :, :, :w] = (
                w1_flat[:, o:o + w].reshape(ET, 128, w).transpose(1, 0, 2))
        # w_out resident: 4 content strips + 2 rope-pair strips
        wo_pack = np.zeros((128, 6, E), f32)
        wog = wo3[hs]                                      # [4, 192, E]
        for kt in range(HPG):
            wo_pack[:, kt, :] = wog[kt, :DH, :]
        for hp in range(2):
            wo_pack[0:RD, 4 + hp, :] = wog[2 * hp, DH:, :]
            wo_pack[RD:128, 4 + hp, :] = wog[2 * hp + 1, DH:, :]
        in_maps.append({
            "xt": xt_pack,
            "w1": w1_pack.astype(NPFP16),
            "wuk": np.ascontiguousarray(
                wup3[:, hs, :DH].reshape(RK, HPG * DH)).astype(NPFP16),
            "wuv": np.ascontiguousarray(
                wup3[:, hs, DH:].reshape(RK, HPG * DV)).astype(NPFP16),
            "wo": wo_pack.astype(NPFP16),
            "cost": cost,
            "sint": sint,
            "jt": jt,
            "triu": triu,
            "ones1": np.ones((1, 128), NPFP16),
        })
    return in_maps


def kernel(x, cos_table, sin_table, wq, wkv_down, w_up, w_out, _want_perf=False):
    if "nc" not in _CACHE:
        _CACHE["nc"] = _build()
    nc = _CACHE["nc"]
    in_maps = _prep_inputs(x, cos_table, sin_table, wq, wkv_down, w_up, w_out)
    res = run_bass_kernel_spmd(nc, in_maps, core_ids=list(range(NCORE)),
                               trace=bool(_want_perf),
                               tmpdir=os.environ.get("BASS_TMPDIR") or None)
    out = np.zeros((B, L, E), np.float32)
    for core in range(NCORE):
        b = core // HPG
        out[b] += res.results[core]["outt"].astype(np.float32)
    if _want_perf:
        return out, res
    return out


# revision 30
# speedup vs baseline: 1.0267x; 1.0189x over previous
"""MLA prefill kernel (fp16) for Trainium2, 8 NeuronCores.

Sharding: data-parallel over batch (2) x tensor-parallel over heads
(16 heads -> 4 per core).  Core c handles batch c//4, head group c%4.
Each core computes its full attention block plus a partial output
projection; the host sums the 4 per-group partials per batch.

All matmul operands are fp16 (1 cycle/row on the PE; fp32/f32r ran in
fp32_mode=HIGH at ~2.5 cycles/row; fp16's 10 mantissa bits keep the
rel-err ~8x below bf16), accumulation stays f32 in PSUM.
Everything is computed transposed ([feature, L]) so matmul lhsT/rhs
operands are produced directly, except V (L-major for the PV matmul),
which stays resident in SBUF.  Scores are computed transposed
(S^T = K Q^T, [Lk, Lq]) so softmax's sum runs through the PV matmul
via an appended ones-column; exp needs no max-subtraction (scores are
O(10)).  RoPE pair mixing runs along partitions via a +-1 pair-swap
matmul (J) plus two elementwise multiplies and an add.

The attention inner loop is software-pipelined two key-tiles ahead
(PV(t) trails QK(t+2)) so the tensor engine does not stall on the exp
activation; causal masks run on the Pool engine.  PSUM: 4 banks hold
the two heads' PV accumulators, 4 banks rotate for scores/projections.
The softmax denominator is inverted as exp(-ln(d)) on the Act engine
and broadcast across partitions with a K=1 ones matmul; each chunk's
normalization tail and output projection are deferred past the next
chunk's QKV so that latency hides behind tensor-engine work.  w_out is
SBUF-resident with the per-head rope halves packed in pairs so the
output projection runs 6 full-K=128 matmuls.
"""

import math
import os
import sys

sys.path.insert(0, "/opt/trn_rl_repo")

import numpy as np

import concourse.bass as bass
import concourse.mybir as mybir
import concourse.tile as tile
from concourse.bass import ds
from concourse.bass_utils import run_bass_kernel_spmd

H, DH, RK, RD = 16, 128, 512, 64
B, L, E = 2, 2048, 2048
HPG = 4                      # heads per core
NCORE = 8
DV = DH + RD                 # 192
SCALE = 1.0 / math.sqrt(DV)
CH = 512                     # Lq chunk
NCH = L // CH                # 4
LT = L // 128                # 16 key tiles
ET = E // 128                # 16
VROW = HPG * (DV + 1)        # 772: per-head 192 v dims + ones col

F32 = mybir.dt.float32
FP16 = mybir.dt.float16
AF = mybir.ActivationFunctionType
NPFP16 = np.float16

_CACHE = {}


def _split_excess_waits(nc, limit=1):
    """walrus on this toolchain accepts at most one sem-wait per
    instruction; hoist extras onto same-engine no-ops just before."""
    f = nc.m.functions[0]
    for bb in f.blocks:
        new_list = []
        changed = False
        for inst in bb.instructions:
            si = inst.sync_info
            if si is not None and si.on_wait is not None and len(si.on_wait) > limit:
                waits = list(si.on_wait)
                changed = True
                n = 0
                while len(waits) > limit:
                    chunk, waits = waits[:limit], waits[limit:]
                    new_list.append(mybir.InstNoOp(
                        name=f"{inst.name}-ws{n}",
                        sync_info=mybir.SyncInfo(on_wait=chunk, on_update=[]),
                        bass_nofuse=True,
                        engine=inst.engine,
                    ))
                    n += 1
                inst.sync_info = mybir.SyncInfo(on_wait=waits, on_update=si.on_update)
            new_list.append(inst)
        if changed:
            bb.instructions[:] = new_list
    return nc


def _build():
    nc = bass.Bass(target_bir_lowering=False, trn_type="TRN2")

    xt = nc.dram_tensor("xt", [NCH, 128, ET, CH], FP16, kind="ExternalInput")
    w1 = nc.dram_tensor("w1", [11, 128, ET, 128], FP16, kind="ExternalInput")
    wuk = nc.dram_tensor("wuk", [RK, HPG * DH], FP16, kind="ExternalInput")
    wuv = nc.dram_tensor("wuv", [RK, HPG * DV], FP16, kind="ExternalInput")
    wo = nc.dram_tensor("wo", [128, 6, E], FP16, kind="ExternalInput")
    cost = nc.dram_tensor("cost", [128, L], FP16, kind="ExternalInput")
    sint = nc.dram_tensor("sint", [128, L], FP16, kind="ExternalInput")
    jt = nc.dram_tensor("jt", [128, 128], FP16, kind="ExternalInput")
    triu = nc.dram_tensor("triu", [128, 128], FP16, kind="ExternalInput")
    ones1 = nc.dram_tensor("ones1", [1, 128], FP16, kind="ExternalInput")
    outt = nc.dram_tensor("outt", [L, E], FP16, kind="ExternalOutput")

    from contextlib import ExitStack

    with tile.TileContext(nc) as tc:
        with ExitStack() as ctx:
            ctx.enter_context(nc.allow_low_precision(
                reason="bf16 kernel; all contractions accumulate in f32 psum"))
            pool_specs = [
                ("consts", 1, None), ("res", 1, None),
                ("xt_p", 2, None), ("w1_p", 4, None),
                ("qt_p", 2, None), ("rq_p", 2, None), ("ckv_p", 2, None),
                ("pt_p", 6, None), ("tmp_p", 2, None),
                ("oz_p", 2, None), ("fin_p", 3, None),
                ("acc_p", 2, "PSUM"), ("rot_p", 4, "PSUM"),
            ]
            pools = {}
            for pname, pbufs, pspace in pool_specs:
                kw = {"name": pname, "bufs": pbufs}
                if pspace:
                    kw["space"] = pspace
                pools[pname] = ctx.enter_context(tc.tile_pool(**kw))
            (consts, res, xt_p, w1_p, qt_p, rq_p, ckv_p, pt_p, tmp_p,
             oz_p, fin_p, acc_p, rot_p) = (pools[s[0]] for s in pool_specs)

            def rot():
                return rot_p.tile([128, 512], F32, tag="ps", name="ps")

            # ---- constants / resident weights.  Only the small consts and
            # cos/sin go ahead of chunk 0's xt/w1 input DMAs; the bulky
            # resident weights (wuk/wuv/wo) are dispatched after chunk 0's
            # QKV emission so the first matmul isn't stuck behind them.
            jt_t = consts.tile([128, 128], FP16, tag="jt", name="jt")
            nc.sync.dma_start(out=jt_t[:], in_=jt.ap())
            tri_t = consts.tile([128, 128], FP16, tag="tri", name="tri")
            nc.sync.dma_start(out=tri_t[:], in_=triu.ap())
            one_t = consts.tile([1, 128], FP16, tag="one", name="one")
            nc.sync.dma_start(out=one_t[:], in_=ones1.ap())
            cos_sb = res.tile([128, L], FP16, tag="cos", name="cos")
            sin_sb = res.tile([128, L], FP16, tag="sin", name="sin")
            wukt = res.tile([128, RK // 128, HPG * DH], FP16, tag="wukt", name="wukt")
            wuvt = res.tile([128, RK // 128, HPG * DV], FP16, tag="wuvt", name="wuvt")
            wo_t = res.tile([128, 6, E], FP16, tag="wo", name="wo")

            def load_residents():
                nc.scalar.dma_start(out=cos_sb[:], in_=cost.ap())
                nc.scalar.dma_start(out=sin_sb[:], in_=sint.ap())
                nc.sync.dma_start(
                    out=wukt[:], in_=wuk.ap().rearrange("(t p) n -> p t n", p=128))
                nc.sync.dma_start(
                    out=wuvt[:], in_=wuv.ap().rearrange("(t p) n -> p t n", p=128))
                nc.scalar.dma_start(out=wo_t[:], in_=wo.ap())

            ktc = res.tile([128, HPG, L], FP16, tag="ktc", name="ktc")   # K content, transposed
            rkd = res.tile([128, L], FP16, tag="rkd", name="rkd")        # roped k_rope, dup rows
            vd = res.tile([128, LT, VROW], FP16, tag="vd", name="vd")    # V resident (L-major + ones)
            vdv = vd[:].rearrange("p t (h x) -> p t h x", x=DV + 1)
            nc.gpsimd.memset(vdv[:, :, :, DV], 1.0)                      # ones columns

            # d-tiles of the fused QKV projection: (kind, idx)
            dtiles = ([("q", i) for i in range(HPG)]
                      + [("ckv", i) for i in range(RK // 128)]
                      + [("rq", i) for i in range(2)]
                      + [("rk", 0)])

            # deferred-work closures (prev chunk's norm tail + output proj),
            # emitted after the next chunk's QKV so the slow reciprocal and
            # the oz writes hide behind tensor-engine work
            pending = []

            for c in range(NCH):
                ccols = ds(c * CH, CH)

                # ================= QKV(c): [1344, CH] = W1^T @ x^T =======
                xtt = xt_p.tile([128, ET, CH], FP16, tag="xtt", name="xtt")
                if c == 0:
                    # sliced load spread across four engine queues: right
                    # after kernel start the DMA fabric ramps slowly per
                    # queue, so parallelize the first transfers
                    eng = [nc.gpsimd, nc.scalar, nc.gpsimd, nc.scalar]
                    for s in range(4):
                        eng[s].dma_start(out=xtt[:, ds(4 * s, 4), :],
                                         in_=xt.ap()[c][:, ds(4 * s, 4), :])
                else:
                    nc.sync.dma_start(out=xtt[:], in_=xt.ap()[c])
                qtc = qt_p.tile([128, HPG, CH], FP16, tag="qtc", name="qtc")
                rq = rq_p.tile([128, 2, CH], FP16, tag="rq", name="rq")
                ckv = ckv_p.tile([128, RK // 128, CH], FP16, tag="ckv", name="ckv")

                for di, (kind, idx) in enumerate(dtiles):
                    w1s = w1_p.tile([128, ET, 128], FP16, tag="w1s", name="w1s")
                    nc.sync.dma_start(out=w1s[:], in_=w1.ap()[di])
                    dw = RD if kind == "rk" else 128
                    ps = rot()
                    for e in range(ET):
                        nc.tensor.matmul(ps[:dw, :CH], w1s[:, e, :dw], xtt[:, e, :],
                                         start=(e == 0), stop=(e == ET - 1))
                    if kind == "q":
                        nc.scalar.copy(out=qtc[:, idx, :], in_=ps[:, :CH])
                    elif kind == "ckv":
                        nc.vector.tensor_copy(ckv[:, idx, :], ps[:, :CH])
                    elif kind == "rq":
                        nc.vector.tensor_copy(rq[:, idx, :], ps[:, :CH])
                    else:  # pre-rope k_rope at partitions 0:64
                        nc.vector.tensor_copy(rkd[0:RD, ccols], ps[:RD, :CH])

                if c == 0:
                    load_residents()
                for fn in pending:
                    fn()
                pending = []

                # ================= RoPE(c) ===============================
                # roped = R * cos + (J @ R) * sin   (pairs along partitions)
                for i in range(2):  # q_rope, two head-pair tiles
                    swp = rot()
                    nc.tensor.matmul(swp[:, :CH], jt_t[:, :], rq[:, i, :],
                                     start=True, stop=True)
                    t1 = tmp_p.tile([128, CH], FP16, tag="ropet", name="ropet")
                    nc.vector.tensor_mul(t1[:], rq[:, i, :], cos_sb[:, ccols])
                    nc.vector.tensor_mul(rq[:, i, :], swp[:, :CH], sin_sb[:, ccols])
                    nc.vector.tensor_add(rq[:, i, :], rq[:, i, :], t1[:])
                swp = rot()
                nc.tensor.matmul(swp[:RD, :CH], jt_t[:RD, :RD], rkd[0:RD, ccols],
                                 start=True, stop=True)
                t1 = tmp_p.tile([128, CH], FP16, tag="ropet", name="ropet")
                nc.vector.tensor_mul(t1[:RD, :], rkd[0:RD, ccols], cos_sb[0:RD, ccols])
                nc.vector.tensor_mul(rkd[0:RD, ccols], swp[:RD, :CH], sin_sb[0:RD, ccols])
                nc.vector.tensor_add(rkd[0:RD, ccols], rkd[0:RD, ccols], t1[:RD, :])
                # duplicate roped k_rope to partitions 64:128 (for odd heads)
                nc.sync.dma_start(out=rkd[RD:128, ccols], in_=rkd[0:RD, ccols])

                # ================= UP-K(c): K^T = Wuk^T @ c_kv^T =========
                for h in range(HPG):
                    ps = rot()
                    for kt in range(RK // 128):
                        nc.tensor.matmul(ps[:, :CH], wukt[:, kt, ds(128 * h, 128)],
                                         ckv[:, kt, :],
                                         start=(kt == 0), stop=(kt == RK // 128 - 1))
                    nc.scalar.copy(out=ktc[:, h, ccols], in_=ps[:, :CH])

                # ================= UP-V(c): V = c_kv @ Wuv (L-major) =====
                for lti in range(4):
                    lt = 4 * c + lti
                    for nb in range(2):
                        psv = rot()
                        for kt in range(RK // 128):
                            nc.tensor.matmul(psv[:, :384],
                                             ckv[:, kt, ds(128 * lti, 128)],
                                             wuvt[:, kt, ds(384 * nb, 384)],
                                             start=(kt == 0), stop=(kt == RK // 128 - 1))
                        for q in range(2):
                            hh = 2 * nb + q
                            nc.vector.tensor_copy(vd[:, lt, ds((DV + 1) * hh, DV)],
                                                  psv[:, ds(DV * q, DV)])

                # ================= ATT(c): head pairs, 1-tile pipelined ==
                ntk = 4 * c + 4
                oz = oz_p.tile([128, 6, CH], FP16, tag="oz", name="oz")

                def attn_half(hp, inject=None):
                    heads = (2 * hp, 2 * hp + 1)
                    A = [(acc_p.tile([128, 512], F32, tag="acc1", name="acc1"),
                          acc_p.tile([128, 512], F32, tag="acc2", name="acc2"))
                         for _ in range(2)]
                    geom = []
                    for t in range(ntk):
                        j = t - 4 * c
                        off = 128 * j if j >= 0 else 0
                        geom.append((off, CH - off, j >= 0))
                    sps_l = {}
                    pt_l = {}

                    def emit_qk(t):
                        off, n, _ = geom[t]
                        sps_l[t] = []
                        for q in range(2):
                            h = heads[q]
                            hb = RD * (h % 2)
                            sps = rot()
                            nc.tensor.matmul(sps[:, ds(off, n)],
                                             ktc[:, h, ds(128 * t, 128)],
                                             qtc[:, h, ds(off, n)],
                                             start=True, stop=False)
                            nc.tensor.matmul(sps[:, ds(off, n)],
                                             rkd[hb:hb + RD, ds(128 * t, 128)],
                                             rq[hb:hb + RD, h // 2, ds(off, n)],
                                             start=False, stop=True)
                            sps_l[t].append(sps)

                    def emit_exp(t):
                        off, n, diag = geom[t]
                        pt_l[t] = []
                        for q in range(2):
                            pt = pt_p.tile([128, CH], FP16, tag="pt", name="pt")
                            nc.scalar.activation(pt[:, ds(off, n)],
                                                 sps_l[t][q][:, ds(off, n)],
                                                 AF.Exp, scale=SCALE)
                            if diag:
                                # on Pool: keeps the mask off the DVE, whose
                                # queue carries the slow reciprocals
                                nc.gpsimd.tensor_mul(pt[:, ds(off, 128)],
                                                     pt[:, ds(off, 128)], tri_t[:])
                            pt_l[t].append(pt)

                    def emit_pv(t):
                        off, n, _ = geom[t]
                        for q in range(2):
                            h = heads[q]
                            ps1, ps2 = A[q]
                            pt = pt_l[t][q]
                            nc.tensor.matmul(ps1[:, ds(off, n)],
                                             vd[:, t, ds((DV + 1) * h, 128)],
                                             pt[:, ds(off, n)],
                                             start=(t == 0), stop=(t == ntk - 1),
                                             skip_group_check=True)
                            nc.tensor.matmul(ps2[:DV - DH + 1, ds(off, n)],
                                             vd[:, t, ds((DV + 1) * h + DH, DV - DH + 1)],
                                             pt[:, ds(off, n)],
                                             start=(t == 0), stop=(t == ntk - 1),
                                             skip_group_check=True)

                    # 2-tile software pipeline: PV(t) trails QK(t+2) so the
                    # exp (and diag mask) latency is fully covered by PE work
                    emit_qk(0)
                    emit_qk(1)
                    emit_exp(0)
                    for t in range(2, ntk):
                        emit_qk(t)
                        emit_pv(t - 2)
                        emit_exp(t - 1)
                        if inject is not None and t == min(3, ntk - 1):
                            inject()
                            inject = None
                    emit_pv(ntk - 2)
                    emit_exp(ntk - 1)
                    emit_pv(ntk - 1)
                    if inject is not None:
                        inject()

                    # softmax denominator reciprocal as exp(-ln(d)) on the
                    # Act engine: ~3x faster than the DVE InstReciprocal and
                    # keeps the DVE free for evacuations
                    rrs = []
                    for q in range(2):
                        lnd = tmp_p.tile([1, CH], F32, tag="lnd", name="lnd",
                                         bufs=4)
                        nc.scalar.activation(lnd[:], A[q][1][RD:RD + 1, :CH],
                                             AF.Ln)
                        rr16 = tmp_p.tile([1, CH], FP16, tag="rr16", name="rr16",
                                          bufs=4)
                        nc.scalar.activation(rr16[:], lnd[:], AF.Exp, scale=-1.0)
                        rrs.append(rr16)

                    def finish_norm():
                        for q in range(2):
                            h = heads[q]
                            ps1, ps2 = A[q]
                            rb = rot()
                            nc.tensor.matmul(rb[:, :CH], one_t[:, :], rrs[q][:, :],
                                             start=True, stop=True)
                            rbs = tmp_p.tile([128, CH], FP16, tag="rbs", name="rbs")
                            nc.vector.tensor_copy(rbs[:], rb[:, :CH])
                            nc.vector.tensor_mul(oz[:, h, :], ps1[:, :CH], rbs[:])
                            hb = RD * (h % 2)
                            nc.vector.tensor_mul(oz[hb:hb + RD, 4 + hp, :],
                                                 ps2[0:RD, :CH], rbs[0:RD, :])
                    return finish_norm

                fin0 = attn_half(0)
                fin1 = attn_half(1, inject=fin0)
                pending.append(fin1)

                # ===== FINAL(c): out = attn @ WO, deferred past QKV(c+1) =
                def make_final(c, oz):
                    def emit_final():
                        dma_eng = [nc.gpsimd, nc.sync, nc.gpsimd, nc.scalar]
                        for eg in range(E // CH):
                            for ls in range(CH // 128):
                                fps = rot()
                                for kt in range(6):
                                    nc.tensor.matmul(fps[:, :CH],
                                                     oz[:, kt, ds(128 * ls, 128)],
                                                     wo_t[:, kt, ds(CH * eg, CH)],
                                                     start=(kt == 0), stop=(kt == 5))
                                fin = fin_p.tile([128, CH], FP16, tag="fin",
                                                 name="fin")
                                # alternate evac + store queues so the last
                                # chunk's drain doesn't serialize on one engine
                                if ls % 2 == 0:
                                    nc.scalar.copy(out=fin[:], in_=fps[:, :CH])
                                else:
                                    nc.vector.tensor_copy(fin[:], fps[:, :CH])
                                dma_eng[ls].dma_start(
                                    out=outt.ap()[ds(c * CH + 128 * ls, 128),
                                                  ds(CH * eg, CH)],
                                    in_=fin[:])
                    return emit_final

                pending.append(make_final(c, oz))

            for fn in pending:
                fn()
            pending = []

    _split_excess_waits(nc)
    return nc


def _prep_inputs(x, cos_table, sin_table, wq, wkv_down, w_up, w_out):
    f32 = np.float32
    wq3 = np.asarray(wq, f32).reshape(E, H, DV)
    wup3 = np.asarray(w_up, f32).reshape(RK, H, 2 * DH + RD)
    wo3 = np.asarray(w_out, f32).reshape(H, DV, E)
    wkv = np.asarray(wkv_down, f32)

    cosI = np.repeat(np.asarray(cos_table, f32)[:L], 2, axis=1).T  # [64, L]
    sinI = np.repeat(np.asarray(sin_table, f32)[:L], 2, axis=1).T
    cost = np.ascontiguousarray(np.concatenate([cosI, cosI], 0)).astype(NPFP16)
    sint = np.ascontiguousarray(np.concatenate([sinI, sinI], 0)).astype(NPFP16)
    J = np.zeros((128, 128), f32)
    for i in range(64):
        J[2 * i, 2 * i + 1] = -1.0
        J[2 * i + 1, 2 * i] = 1.0
    jt = np.ascontiguousarray(J.T).astype(NPFP16)
    triu = np.ascontiguousarray(np.triu(np.ones((128, 128), f32))).astype(NPFP16)

    in_maps = []
    for core in range(NCORE):
        b, g = core // HPG, core % HPG
        hs = slice(HPG * g, HPG * g + HPG)
        xT = np.asarray(x, f32)[b].T                       # [E, L]
        xt_pack = np.ascontiguousarray(
            xT.reshape(ET, 128, NCH, CH).transpose(2, 1, 0, 3)).astype(NPFP16)
        wq_c = wq3[:, hs, :DH].reshape(E, HPG * DH)
        wq_r = wq3[:, hs, DH:].reshape(E, HPG * RD)
        w1_flat = np.concatenate([wq_c, wkv[:, :RK], wq_r, wkv[:, RK:]], axis=1)
        # pack into 11 d-strips [128, ET, 128] (last strip: 64 cols, zero-pad)
        w1_pack = np.zeros((11, 128, ET, 128), f32)
        offs = [128 * i for i in range(10)] + [1280]
        wids = [128] * 10 + [64]
        for di, (o, w) in enumerate(zip(offs, wids)):
            w1_pack[di, :, :, :w] = (
                w1_flat[:, o:o + w].reshape(ET, 128, w).transpose(1, 0, 2))
        # w_out resident: 4 content strips + 2 rope-pair strips
        wo_pack = np.zeros((128, 6, E), f32)
        wog = wo3[hs]                                      # [4, 192, E]
        for kt in range(HPG):
            wo_pack[:, kt, :] = wog[kt, :DH, :]
        for hp in range(2):
            wo_pack[0:RD, 4 + hp, :] = wog[2 * hp, DH:, :]
            wo_pack[RD:128, 4 + hp, :] = wog[2 * hp + 1, DH:, :]
        in_maps.append({
            "xt": xt_pack,
            "w1": w1_pack.astype(NPFP16),
            "wuk": np.ascontiguousarray(
                wup3[:, hs, :DH].reshape(RK, HPG * DH)).astype(NPFP16),
            "wuv": np.ascontiguousarray(
                wup3[:, hs, DH:].reshape(RK, HPG * DV)).astype(NPFP16),
            "wo": wo_pack.astype(NPFP16),
            "cost": cost,
            "sint": sint,
            "jt": jt,
            "triu": triu,
            "ones1": np.ones((1, 128), NPFP16),
        })
    return in_maps


def kernel(x, cos_table, sin_table, wq, wkv_down, w_up, w_out, _want_perf=False):
    if "nc" not in _CACHE:
        _CACHE["nc"] = _build()
    nc = _CACHE["nc"]
    in_maps = _prep_inputs(x, cos_table, sin_table, wq, wkv_down, w_up, w_out)
    res = run_bass_kernel_spmd(nc, in_maps, core_ids=list(range(NCORE)),
                               trace=bool(_want_perf),
                               tmpdir=os.environ.get("BASS_TMPDIR") or None)
    out = np.zeros((B, L, E), np.float32)
    for core in range(NCORE):
        b = core // HPG
        out[b] += res.results[core]["outt"].astype(np.float32)
    if _want_perf:
        return out, res
    return out


# revision 31
# speedup vs baseline: 1.0443x; 1.0171x over previous
"""MLA prefill kernel (fp16) for Trainium2, 8 NeuronCores.

Sharding: data-parallel over batch (2) x tensor-parallel over heads
(16 heads -> 4 per core).  Core c handles batch c//4, head group c%4.
Each core computes its full attention block plus a partial output
projection; the host sums the 4 per-group partials per batch.

All matmul operands are fp16 (1 cycle/row on the PE; fp32/f32r ran in
fp32_mode=HIGH at ~2.5 cycles/row; fp16's 10 mantissa bits keep the
rel-err ~8x below bf16), accumulation stays f32 in PSUM.
Everything is computed transposed ([feature, L]) so matmul lhsT/rhs
operands are produced directly, except V (L-major for the PV matmul),
which stays resident in SBUF.  Scores are computed transposed
(S^T = K Q^T, [Lk, Lq]) so softmax's sum runs through the PV matmul
via an appended ones-column; exp needs no max-subtraction (scores are
O(10)).  RoPE pair mixing runs along partitions via a +-1 pair-swap
matmul (J) plus two elementwise multiplies and an add.

The attention inner loop is software-pipelined two key-tiles ahead
(PV(t) trails QK(t+2)) so the tensor engine does not stall on the exp
activation; causal masks run on the Pool engine.  PSUM: 4 banks hold
the two heads' PV accumulators, 4 banks rotate for scores/projections.
The softmax denominator is inverted as exp(-ln(d)) on the Act engine
and broadcast across partitions with a K=1 ones matmul; each chunk's
normalization tail and output projection are deferred past the next
chunk's QKV so that latency hides behind tensor-engine work.  w_out is
SBUF-resident with the per-head rope halves packed in pairs so the
output projection runs 6 full-K=128 matmuls.
"""

import math
import os
import sys

sys.path.insert(0, "/opt/trn_rl_repo")

import numpy as np

import concourse.bass as bass
import concourse.mybir as mybir
import concourse.tile as tile
from concourse.bass import ds
from concourse.bass_utils import run_bass_kernel_spmd

H, DH, RK, RD = 16, 128, 512, 64
B, L, E = 2, 2048, 2048
HPG = 4                      # heads per core
NCORE = 8
DV = DH + RD                 # 192
SCALE = 1.0 / math.sqrt(DV)
CH = 512                     # Lq chunk
NCH = L // CH                # 4
LT = L // 128                # 16 key tiles
ET = E // 128                # 16
VROW = HPG * (DV + 1)        # 772: per-head 192 v dims + ones col

F32 = mybir.dt.float32
FP16 = mybir.dt.float16
AF = mybir.ActivationFunctionType
NPFP16 = np.float16

_CACHE = {}


def _split_excess_waits(nc, limit=1):
    """walrus on this toolchain accepts at most one sem-wait per
    instruction; hoist extras onto same-engine no-ops just before."""
    f = nc.m.functions[0]
    for bb in f.blocks:
        new_list = []
        changed = False
        for inst in bb.instructions:
            si = inst.sync_info
            if si is not None and si.on_wait is not None and len(si.on_wait) > limit:
                waits = list(si.on_wait)
                changed = True
                n = 0
                while len(waits) > limit:
                    chunk, waits = waits[:limit], waits[limit:]
                    new_list.append(mybir.InstNoOp(
                        name=f"{inst.name}-ws{n}",
                        sync_info=mybir.SyncInfo(on_wait=chunk, on_update=[]),
                        bass_nofuse=True,
                        engine=inst.engine,
                    ))
                    n += 1
                inst.sync_info = mybir.SyncInfo(on_wait=waits, on_update=si.on_update)
            new_list.append(inst)
        if changed:
            bb.instructions[:] = new_list
    return nc


def _build():
    nc = bass.Bass(target_bir_lowering=False, trn_type="TRN2")

    xt = nc.dram_tensor("xt", [NCH, 128, ET, CH], FP16, kind="ExternalInput")
    w1 = nc.dram_tensor("w1", [11, 128, ET, 128], FP16, kind="ExternalInput")
    wuk = nc.dram_tensor("wuk", [RK, HPG * DH], FP16, kind="ExternalInput")
    wuv = nc.dram_tensor("wuv", [RK, HPG * DV], FP16, kind="ExternalInput")
    wo = nc.dram_tensor("wo", [128, 6, E], FP16, kind="ExternalInput")
    cost = nc.dram_tensor("cost", [128, L], FP16, kind="ExternalInput")
    sint = nc.dram_tensor("sint", [128, L], FP16, kind="ExternalInput")
    jt = nc.dram_tensor("jt", [128, 128], FP16, kind="ExternalInput")
    triu = nc.dram_tensor("triu", [128, 128], FP16, kind="ExternalInput")
    ones1 = nc.dram_tensor("ones1", [1, 128], FP16, kind="ExternalInput")
    outt = nc.dram_tensor("outt", [L, E], FP16, kind="ExternalOutput")

    from contextlib import ExitStack

    with tile.TileContext(nc) as tc:
        with ExitStack() as ctx:
            ctx.enter_context(nc.allow_low_precision(
                reason="bf16 kernel; all contractions accumulate in f32 psum"))
            pool_specs = [
                ("consts", 1, None), ("res", 1, None),
                ("xt_p", 2, None), ("w1_p", 4, None),
                ("qt_p", 2, None), ("rq_p", 2, None), ("ckv_p", 2, None),
                ("pt_p", 6, None), ("tmp_p", 2, None),
                ("oz_p", 2, None), ("fin_p", 3, None),
                ("acc_p", 2, "PSUM"), ("rot_p", 4, "PSUM"),
            ]
            pools = {}
            for pname, pbufs, pspace in pool_specs:
                kw = {"name": pname, "bufs": pbufs}
                if pspace:
                    kw["space"] = pspace
                pools[pname] = ctx.enter_context(tc.tile_pool(**kw))
            (consts, res, xt_p, w1_p, qt_p, rq_p, ckv_p, pt_p, tmp_p,
             oz_p, fin_p, acc_p, rot_p) = (pools[s[0]] for s in pool_specs)

            def rot():
                return rot_p.tile([128, 512], F32, tag="ps", name="ps")

            # ---- constants / resident weights.  Only the small consts and
            # cos/sin go ahead of chunk 0's xt/w1 input DMAs; the bulky
            # resident weights (wuk/wuv/wo) are dispatched after chunk 0's
            # QKV emission so the first matmul isn't stuck behind them.
            jt_t = consts.tile([128, 128], FP16, tag="jt", name="jt")
            nc.sync.dma_start(out=jt_t[:], in_=jt.ap())
            tri_t = consts.tile([128, 128], FP16, tag="tri", name="tri")
            nc.sync.dma_start(out=tri_t[:], in_=triu.ap())
            one_t = consts.tile([1, 128], FP16, tag="one", name="one")
            nc.sync.dma_start(out=one_t[:], in_=ones1.ap())
            cos_sb = res.tile([128, L], FP16, tag="cos", name="cos")
            sin_sb = res.tile([128, L], FP16, tag="sin", name="sin")
            wukt = res.tile([128, RK // 128, HPG * DH], FP16, tag="wukt", name="wukt")
            wuvt = res.tile([128, RK // 128, HPG * DV], FP16, tag="wuvt", name="wuvt")
            wo_t = res.tile([128, 6, E], FP16, tag="wo", name="wo")

            def load_residents():
                nc.scalar.dma_start(out=cos_sb[:], in_=cost.ap())
                nc.scalar.dma_start(out=sin_sb[:], in_=sint.ap())
                nc.sync.dma_start(
                    out=wukt[:], in_=wuk.ap().rearrange("(t p) n -> p t n", p=128))
                nc.sync.dma_start(
                    out=wuvt[:], in_=wuv.ap().rearrange("(t p) n -> p t n", p=128))
                nc.scalar.dma_start(out=wo_t[:], in_=wo.ap())

            ktc = res.tile([128, HPG, L], FP16, tag="ktc", name="ktc")   # K content, transposed
            rkd = res.tile([128, L], FP16, tag="rkd", name="rkd")        # roped k_rope, dup rows
            vd = res.tile([128, LT, VROW], FP16, tag="vd", name="vd")    # V resident (L-major + ones)
            vdv = vd[:].rearrange("p t (h x) -> p t h x", x=DV + 1)
            nc.gpsimd.memset(vdv[:, :, :, DV], 1.0)                      # ones columns

            # d-tiles of the fused QKV projection: (kind, idx)
            dtiles = ([("q", i) for i in range(HPG)]
                      + [("ckv", i) for i in range(RK // 128)]
                      + [("rq", i) for i in range(2)]
                      + [("rk", 0)])

            # deferred-work closures (prev chunk's norm tail + output proj),
            # emitted after the next chunk's QKV so the slow reciprocal and
            # the oz writes hide behind tensor-engine work
            pending = []

            for c in range(NCH):
                ccols = ds(c * CH, CH)

                # ================= QKV(c): [1344, CH] = W1^T @ x^T =======
                xtt = xt_p.tile([128, ET, CH], FP16, tag="xtt", name="xtt")
                if c == 0:
                    # sliced load spread across four engine queues: right
                    # after kernel start the DMA fabric ramps slowly per
                    # queue, so parallelize the first transfers
                    eng = [nc.gpsimd, nc.scalar, nc.gpsimd, nc.scalar]
                    for s in range(4):
                        eng[s].dma_start(out=xtt[:, ds(4 * s, 4), :],
                                         in_=xt.ap()[c][:, ds(4 * s, 4), :])
                else:
                    nc.sync.dma_start(out=xtt[:], in_=xt.ap()[c])
                qtc = qt_p.tile([128, HPG, CH], FP16, tag="qtc", name="qtc")
                rq = rq_p.tile([128, 2, CH], FP16, tag="rq", name="rq")
                ckv = ckv_p.tile([128, RK // 128, CH], FP16, tag="ckv", name="ckv")

                for di, (kind, idx) in enumerate(dtiles):
                    w1s = w1_p.tile([128, ET, 128], FP16, tag="w1s", name="w1s")
                    nc.sync.dma_start(out=w1s[:], in_=w1.ap()[di])
                    dw = RD if kind == "rk" else 128
                    ps = rot()
                    for e in range(ET):
                        nc.tensor.matmul(ps[:dw, :CH], w1s[:, e, :dw], xtt[:, e, :],
                                         start=(e == 0), stop=(e == ET - 1))
                    if kind == "q":
                        nc.scalar.copy(out=qtc[:, idx, :], in_=ps[:, :CH])
                    elif kind == "ckv":
                        nc.vector.tensor_copy(ckv[:, idx, :], ps[:, :CH])
                    elif kind == "rq":
                        nc.vector.tensor_copy(rq[:, idx, :], ps[:, :CH])
                    else:  # pre-rope k_rope at partitions 0:64
                        nc.vector.tensor_copy(rkd[0:RD, ccols], ps[:RD, :CH])

                if c == 0:
                    load_residents()
                for fn in pending:
                    fn()
                pending = []

                # ================= RoPE(c) ===============================
                # roped = R * cos + (J @ R) * sin   (pairs along partitions)
                for i in range(2):  # q_rope, two head-pair tiles
                    swp = rot()
                    nc.tensor.matmul(swp[:, :CH], jt_t[:, :], rq[:, i, :],
                                     start=True, stop=True)
                    t1 = tmp_p.tile([128, CH], FP16, tag="ropet", name="ropet")
                    nc.vector.tensor_mul(t1[:], rq[:, i, :], cos_sb[:, ccols])
                    nc.vector.tensor_mul(rq[:, i, :], swp[:, :CH], sin_sb[:, ccols])
                    nc.vector.tensor_add(rq[:, i, :], rq[:, i, :], t1[:])
                swp = rot()
                nc.tensor.matmul(swp[:RD, :CH], jt_t[:RD, :RD], rkd[0:RD, ccols],
                                 start=True, stop=True)
                t1 = tmp_p.tile([128, CH], FP16, tag="ropet", name="ropet")
                nc.vector.tensor_mul(t1[:RD, :], rkd[0:RD, ccols], cos_sb[0:RD, ccols])
                nc.vector.tensor_mul(rkd[0:RD, ccols], swp[:RD, :CH], sin_sb[0:RD, ccols])
                nc.vector.tensor_add(rkd[0:RD, ccols], rkd[0:RD, ccols], t1[:RD, :])
                # duplicate roped k_rope to partitions 64:128 (for odd heads)
                nc.sync.dma_start(out=rkd[RD:128, ccols], in_=rkd[0:RD, ccols])

                # ================= UP-K(c): K^T = Wuk^T @ c_kv^T =========
                for h in range(HPG):
                    ps = rot()
                    for kt in range(RK // 128):
                        nc.tensor.matmul(ps[:, :CH], wukt[:, kt, ds(128 * h, 128)],
                                         ckv[:, kt, :],
                                         start=(kt == 0), stop=(kt == RK // 128 - 1))
                    nc.scalar.copy(out=ktc[:, h, ccols], in_=ps[:, :CH])

                # ================= UP-V(c): V = c_kv @ Wuv (L-major) =====
                for lti in range(4):
                    lt = 4 * c + lti
                    for nb in range(2):
                        psv = rot()
                        for kt in range(RK // 128):
                            nc.tensor.matmul(psv[:, :384],
                                             ckv[:, kt, ds(128 * lti, 128)],
                                             wuvt[:, kt, ds(384 * nb, 384)],
                                             start=(kt == 0), stop=(kt == RK // 128 - 1))
                        for q in range(2):
                            hh = 2 * nb + q
                            nc.vector.tensor_copy(vd[:, lt, ds((DV + 1) * hh, DV)],
                                                  psv[:, ds(DV * q, DV)])

                # ================= ATT(c): head pairs, 1-tile pipelined ==
                ntk = 4 * c + 4
                oz = oz_p.tile([128, 6, CH], FP16, tag="oz", name="oz")

                def attn_half(hp, inject=None):
                    heads = (2 * hp, 2 * hp + 1)
                    A = [(acc_p.tile([128, 512], F32, tag="acc1", name="acc1"),
                          acc_p.tile([128, 512], F32, tag="acc2", name="acc2"))
                         for _ in range(2)]
                    geom = []
                    for t in range(ntk):
                        j = t - 4 * c
                        off = 128 * j if j >= 0 else 0
                        geom.append((off, CH - off, j >= 0))
                    sps_l = {}
                    pt_l = {}

                    def emit_qk(t):
                        off, n, _ = geom[t]
                        sps_l[t] = []
                        for q in range(2):
                            h = heads[q]
                            hb = RD * (h % 2)
                            sps = rot()
                            nc.tensor.matmul(sps[:, ds(off, n)],
                                             ktc[:, h, ds(128 * t, 128)],
                                             qtc[:, h, ds(off, n)],
                                             start=True, stop=False)
                            nc.tensor.matmul(sps[:, ds(off, n)],
                                             rkd[hb:hb + RD, ds(128 * t, 128)],
                                             rq[hb:hb + RD, h // 2, ds(off, n)],
                                             start=False, stop=True)
                            sps_l[t].append(sps)

                    def emit_exp(t):
                        off, n, diag = geom[t]
                        pt_l[t] = []
                        for q in range(2):
                            pt = pt_p.tile([128, CH], FP16, tag="pt", name="pt")
                            nc.scalar.activation(pt[:, ds(off, n)],
                                                 sps_l[t][q][:, ds(off, n)],
                                                 AF.Exp, scale=SCALE)
                            if diag:
                                # on Pool: keeps the mask off the DVE, whose
                                # queue carries the slow reciprocals
                                nc.gpsimd.tensor_mul(pt[:, ds(off, 128)],
                                                     pt[:, ds(off, 128)], tri_t[:])
                            pt_l[t].append(pt)

                    def emit_pv(t):
                        off, n, _ = geom[t]
                        for q in range(2):
                            h = heads[q]
                            ps1, ps2 = A[q]
                            pt = pt_l[t][q]
                            nc.tensor.matmul(ps1[:, ds(off, n)],
                                             vd[:, t, ds((DV + 1) * h, 128)],
                                             pt[:, ds(off, n)],
                                             start=(t == 0), stop=(t == ntk - 1),
                                             skip_group_check=True)
                            nc.tensor.matmul(ps2[:DV - DH + 1, ds(off, n)],
                                             vd[:, t, ds((DV + 1) * h + DH, DV - DH + 1)],
                                             pt[:, ds(off, n)],
                                             start=(t == 0), stop=(t == ntk - 1),
                                             skip_group_check=True)

                    # 2-tile software pipeline: PV(t) trails QK(t+2) so the
                    # exp (and diag mask) latency is fully covered by PE work
                    emit_qk(0)
                    emit_qk(1)
                    emit_exp(0)
                    for t in range(2, ntk):
                        emit_qk(t)
                        emit_pv(t - 2)
                        emit_exp(t - 1)
                        if inject is not None and t == min(3, ntk - 1):
                            inject()
                            inject = None
                    emit_pv(ntk - 2)
                    emit_exp(ntk - 1)
                    emit_pv(ntk - 1)
                    if inject is not None:
                        inject()

                    # softmax denominator reciprocal as exp(-ln(d)) on the
                    # Act engine: ~3x faster than the DVE InstReciprocal and
                    # keeps the DVE free for evacuations
                    rrs = []
                    for q in range(2):
                        lnd = tmp_p.tile([1, CH], F32, tag="lnd", name="lnd",
                                         bufs=4)
                        nc.scalar.activation(lnd[:], A[q][1][RD:RD + 1, :CH],
                                             AF.Ln)
                        rr16 = tmp_p.tile([1, CH], FP16, tag="rr16", name="rr16",
                                          bufs=4)
                        nc.scalar.activation(rr16[:], lnd[:], AF.Exp, scale=-1.0)
                        rrs.append(rr16)

                    def finish_norm():
                        for q in range(2):
                            h = heads[q]
                            ps1, ps2 = A[q]
                            rb = rot()
                            nc.tensor.matmul(rb[:, :CH], one_t[:, :], rrs[q][:, :],
                                             start=True, stop=True)
                            rbs = tmp_p.tile([128, CH], FP16, tag="rbs", name="rbs")
                            nc.vector.tensor_copy(rbs[:], rb[:, :CH])
                            nc.vector.tensor_mul(oz[:, h, :], ps1[:, :CH], rbs[:])
                            hb = RD * (h % 2)
                            nc.vector.tensor_mul(oz[hb:hb + RD, 4 + hp, :],
                                                 ps2[0:RD, :CH], rbs[0:RD, :])
                    return finish_norm

                fin0 = attn_half(0)
                fin1 = attn_half(1, inject=fin0)
                pending.append(fin1)

                # ===== FINAL(c): out = attn @ WO, deferred past QKV(c+1) =
                def make_final(c, oz):
                    def emit_final():
                        # keep Pool free during attention: its queue runs the
                        # causal masks, which sit on the exp->PV critical path
                        dma_eng = [nc.sync, nc.scalar, nc.sync, nc.scalar]
                        for eg in range(E // CH):
                            for ls in range(CH // 128):
                                fps = rot()
                                for kt in range(6):
                                    nc.tensor.matmul(fps[:, :CH],
                                                     oz[:, kt, ds(128 * ls, 128)],
                                                     wo_t[:, kt, ds(CH * eg, CH)],
                                                     start=(kt == 0), stop=(kt == 5))
                                fin = fin_p.tile([128, CH], FP16, tag="fin",
                                                 name="fin")
                                # alternate evac + store queues so the last
                                # chunk's drain doesn't serialize on one engine
                                if ls % 2 == 0:
                                    nc.scalar.copy(out=fin[:], in_=fps[:, :CH])
                                else:
                                    nc.vector.tensor_copy(fin[:], fps[:, :CH])
                                dma_eng[ls].dma_start(
                                    out=outt.ap()[ds(c * CH + 128 * ls, 128),
                                                  ds(CH * eg, CH)],
                                    in_=fin[:])
                    return emit_final

                pending.append(make_final(c, oz))

            for fn in pending:
                fn()
            pending = []

    _split_excess_waits(nc)
    return nc


def _prep_inputs(x, cos_table, sin_table, wq, wkv_down, w_up, w_out):
    f32 = np.float32
    wq3 = np.asarray(wq, f32).reshape(E, H, DV)
    wup3 = np.asarray(w_up, f32).reshape(RK, H, 2 * DH + RD)
    wo3 = np.asarray(w_out, f32).reshape(H, DV, E)
    wkv = np.asarray(wkv_down, f32)

    cosI = np.repeat(np.asarray(cos_table, f32)[:L], 2, axis=1).T  # [64, L]
    sinI = np.repeat(np.asarray(sin_table, f32)[:L], 2, axis=1).T
    cost = np.ascontiguousarray(np.concatenate([cosI, cosI], 0)).astype(NPFP16)
    sint = np.ascontiguousarray(np.concatenate([sinI, sinI], 0)).astype(NPFP16)
    J = np.zeros((128, 128), f32)
    for i in range(64):
        J[2 * i, 2 * i + 1] = -1.0
        J[2 * i + 1, 2 * i] = 1.0
    jt = np.ascontiguousarray(J.T).astype(NPFP16)
    triu = np.ascontiguousarray(np.triu(np.ones((128, 128), f32))).astype(NPFP16)

    in_maps = []
    for core in range(NCORE):
        b, g = core // HPG, core % HPG
        hs = slice(HPG * g, HPG * g + HPG)
        xT = np.asarray(x, f32)[b].T                       # [E, L]
        xt_pack = np.ascontiguousarray(
            xT.reshape(ET, 128, NCH, CH).transpose(2, 1, 0, 3)).astype(NPFP16)
        wq_c = wq3[:, hs, :DH].reshape(E, HPG * DH)
        wq_r = wq3[:, hs, DH:].reshape(E, HPG * RD)
        w1_flat = np.concatenate([wq_c, wkv[:, :RK], wq_r, wkv[:, RK:]], axis=1)
        # pack into 11 d-strips [128, ET, 128] (last strip: 64 cols, zero-pad)
        w1_pack = np.zeros((11, 128, ET, 128), f32)
        offs = [128 * i for i in range(10)] + [1280]
        wids = [128] * 10 + [64]
        for di, (o, w) in enumerate(zip(offs, wids)):
            w1_pack[di, :, :, :w] = (
                w1_flat[:, o:o + w].reshape(ET, 128, w).transpose(1, 0, 2))
        # w_out resident: 4 content strips + 2 rope-pair strips
        wo_pack = np.zeros((128, 6, E), f32)
        wog = wo3[hs]                                      # [4, 192, E]
        for kt in range(HPG):
            wo_pack[:, kt, :] = wog[kt, :DH, :]
        for hp in range(2):
            wo_pack[0:RD, 4 + hp, :] = wog[2 * hp, DH:, :]
            wo_pack[RD:128, 4 + hp, :] = wog[2 * hp + 1, DH:, :]
        in_maps.append({
            "xt": xt_pack,
            "w1": w1_pack.astype(NPFP16),
            "wuk": np.ascontiguousarray(
                wup3[:, hs, :DH].reshape(RK, HPG * DH)).astype(NPFP16),
            "wuv": np.ascontiguousarray(
                wup3[:, hs, DH:].reshape(RK, HPG * DV)).astype(NPFP16),
            "wo": wo_pack.astype(NPFP16),
            "cost": cost,
            "sint": sint,
            "jt": jt,
            "triu": triu,
            "ones1": np.ones((1, 128), NPFP16),
        })
    return in_maps


def kernel(x, cos_table, sin_table, wq, wkv_down, w_up, w_out, _want_perf=False):
    if "nc" not in _CACHE:
        _CACHE["nc"] = _build()
    nc = _CACHE["nc"]
    in_maps = _prep_inputs(x, cos_table, sin_table, wq, wkv_down, w_up, w_out)
    res = run_bass_kernel_spmd(nc, in_maps, core_ids=list(range(NCORE)),
                               trace=bool(_want_perf),
                               tmpdir=os.environ.get("BASS_TMPDIR") or None)
    out = np.zeros((B, L, E), np.float32)
    for core in range(NCORE):
        b = core // HPG
        out[b] += res.results[core]["outt"].astype(np.float32)
    if _want_perf:
        return out, res
    return out


# revision 33
# speedup vs baseline: 1.0463x; 1.0020x over previous
"""MLA prefill kernel (fp16) for Trainium2, 8 NeuronCores.

Sharding: data-parallel over batch (2) x tensor-parallel over heads
(16 heads -> 4 per core).  Core c handles batch c//4, head group c%4.
Each core computes its full attention block plus a partial output
projection; the host sums the 4 per-group partials per batch.

All matmul operands are fp16 (1 cycle/row on the PE; fp32/f32r ran in
fp32_mode=HIGH at ~2.5 cycles/row; fp16's 10 mantissa bits keep the
rel-err ~8x below bf16), accumulation stays f32 in PSUM.
Everything is computed transposed ([feature, L]) so matmul lhsT/rhs
operands are produced directly, except V (L-major for the PV matmul),
which stays resident in SBUF.  Scores are computed transposed
(S^T = K Q^T, [Lk, Lq]) so softmax's sum runs through the PV matmul
via an appended ones-column; exp needs no max-subtraction (scores are
O(10)).  RoPE pair mixing runs along partitions via a +-1 pair-swap
matmul (J) plus two elementwise multiplies and an add.

The attention inner loop is software-pipelined two key-tiles ahead
(PV(t) trails QK(t+2)) so the tensor engine does not stall on the exp
activation; causal masks run on the Pool engine.  PSUM: 4 banks hold
the two heads' PV accumulators, 4 banks rotate for scores/projections.
The softmax denominator is inverted as exp(-ln(d)) on the Act engine
and broadcast across partitions with a K=1 ones matmul; each chunk's
normalization tail and output projection are deferred past the next
chunk's QKV so that latency hides behind tensor-engine work.  w_out is
SBUF-resident with the per-head rope halves packed in pairs so the
output projection runs 6 full-K=128 matmuls.
"""

import math
import os
import sys

sys.path.insert(0, "/opt/trn_rl_repo")

import numpy as np

import concourse.bass as bass
import concourse.mybir as mybir
import concourse.tile as tile
from concourse.bass import ds
from concourse.bass_utils import run_bass_kernel_spmd

H, DH, RK, RD = 16, 128, 512, 64
B, L, E = 2, 2048, 2048
HPG = 4                      # heads per core
NCORE = 8
DV = DH + RD                 # 192
SCALE = 1.0 / math.sqrt(DV)
CH = 512                     # Lq chunk
NCH = L // CH                # 4
LT = L // 128                # 16 key tiles
ET = E // 128                # 16
VROW = HPG * (DV + 1)        # 772: per-head 192 v dims + ones col

F32 = mybir.dt.float32
FP16 = mybir.dt.float16
AF = mybir.ActivationFunctionType
NPFP16 = np.float16

_CACHE = {}


def _split_excess_waits(nc, limit=1):
    """walrus on this toolchain accepts at most one sem-wait per
    instruction; hoist extras onto same-engine no-ops just before."""
    f = nc.m.functions[0]
    for bb in f.blocks:
        new_list = []
        changed = False
        for inst in bb.instructions:
            si = inst.sync_info
            if si is not None and si.on_wait is not None and len(si.on_wait) > limit:
                waits = list(si.on_wait)
                changed = True
                n = 0
                while len(waits) > limit:
                    chunk, waits = waits[:limit], waits[limit:]
                    new_list.append(mybir.InstNoOp(
                        name=f"{inst.name}-ws{n}",
                        sync_info=mybir.SyncInfo(on_wait=chunk, on_update=[]),
                        bass_nofuse=True,
                        engine=inst.engine,
                    ))
                    n += 1
                inst.sync_info = mybir.SyncInfo(on_wait=waits, on_update=si.on_update)
            new_list.append(inst)
        if changed:
            bb.instructions[:] = new_list
    return nc


def _build():
    nc = bass.Bass(target_bir_lowering=False, trn_type="TRN2")

    xt = nc.dram_tensor("xt", [NCH, 128, ET, CH], FP16, kind="ExternalInput")
    w1 = nc.dram_tensor("w1", [11, 128, ET, 128], FP16, kind="ExternalInput")
    wuk = nc.dram_tensor("wuk", [RK, HPG * DH], FP16, kind="ExternalInput")
    wuv = nc.dram_tensor("wuv", [RK, HPG * DV], FP16, kind="ExternalInput")
    wo = nc.dram_tensor("wo", [128, 6, E], FP16, kind="ExternalInput")
    cost = nc.dram_tensor("cost", [128, L], FP16, kind="ExternalInput")
    sint = nc.dram_tensor("sint", [128, L], FP16, kind="ExternalInput")
    jt = nc.dram_tensor("jt", [128, 128], FP16, kind="ExternalInput")
    triu = nc.dram_tensor("triu", [128, 128], FP16, kind="ExternalInput")
    ones1 = nc.dram_tensor("ones1", [1, 128], FP16, kind="ExternalInput")
    outt = nc.dram_tensor("outt", [L, E], FP16, kind="ExternalOutput")

    from contextlib import ExitStack

    with tile.TileContext(nc) as tc:
        with ExitStack() as ctx:
            ctx.enter_context(nc.allow_low_precision(
                reason="bf16 kernel; all contractions accumulate in f32 psum"))
            pool_specs = [
                ("consts", 1, None), ("res", 1, None),
                ("xt_p", 2, None), ("w1_p", 4, None),
                ("qt_p", 2, None), ("rq_p", 2, None), ("ckv_p", 2, None),
                ("pt_p", 6, None), ("tmp_p", 2, None),
                ("oz_p", 2, None), ("fin_p", 3, None),
                ("acc_p", 2, "PSUM"), ("rot_p", 4, "PSUM"),
            ]
            pools = {}
            for pname, pbufs, pspace in pool_specs:
                kw = {"name": pname, "bufs": pbufs}
                if pspace:
                    kw["space"] = pspace
                pools[pname] = ctx.enter_context(tc.tile_pool(**kw))
            (consts, res, xt_p, w1_p, qt_p, rq_p, ckv_p, pt_p, tmp_p,
             oz_p, fin_p, acc_p, rot_p) = (pools[s[0]] for s in pool_specs)

            def rot():
                return rot_p.tile([128, 512], F32, tag="ps", name="ps")

            # ---- constants / resident weights.  Only the small consts and
            # cos/sin go ahead of chunk 0's xt/w1 input DMAs; the bulky
            # resident weights (wuk/wuv/wo) are dispatched after chunk 0's
            # QKV emission so the first matmul isn't stuck behind them.
            jt_t = consts.tile([128, 128], FP16, tag="jt", name="jt")
            nc.sync.dma_start(out=jt_t[:], in_=jt.ap())
            tri_t = consts.tile([128, 128], FP16, tag="tri", name="tri")
            nc.sync.dma_start(out=tri_t[:], in_=triu.ap())
            one_t = consts.tile([1, 128], FP16, tag="one", name="one")
            nc.sync.dma_start(out=one_t[:], in_=ones1.ap())
            cos_sb = res.tile([128, L], FP16, tag="cos", name="cos")
            sin_sb = res.tile([128, L], FP16, tag="sin", name="sin")
            wukt = res.tile([128, RK // 128, HPG * DH], FP16, tag="wukt", name="wukt")
            wuvt = res.tile([128, RK // 128, HPG * DV], FP16, tag="wuvt", name="wuvt")
            wo_t = res.tile([128, 6, E], FP16, tag="wo", name="wo")

            def load_residents():
                nc.scalar.dma_start(out=cos_sb[:], in_=cost.ap())
                nc.scalar.dma_start(out=sin_sb[:], in_=sint.ap())
                nc.sync.dma_start(
                    out=wukt[:], in_=wuk.ap().rearrange("(t p) n -> p t n", p=128))
                nc.sync.dma_start(
                    out=wuvt[:], in_=wuv.ap().rearrange("(t p) n -> p t n", p=128))
                nc.scalar.dma_start(out=wo_t[:], in_=wo.ap())

            ktc = res.tile([128, HPG, L], FP16, tag="ktc", name="ktc")   # K content, transposed
            rkd = res.tile([128, L], FP16, tag="rkd", name="rkd")        # roped k_rope, dup rows
            vd = res.tile([128, LT, VROW], FP16, tag="vd", name="vd")    # V resident (L-major + ones)
            vdv = vd[:].rearrange("p t (h x) -> p t h x", x=DV + 1)
            nc.gpsimd.memset(vdv[:, :, :, DV], 1.0)                      # ones columns

            # d-tiles of the fused QKV projection: (kind, idx)
            dtiles = ([("q", i) for i in range(HPG)]
                      + [("ckv", i) for i in range(RK // 128)]
                      + [("rq", i) for i in range(2)]
                      + [("rk", 0)])

            # deferred-work closures (prev chunk's norm tail + output proj),
            # emitted after the next chunk's QKV so the slow reciprocal and
            # the oz writes hide behind tensor-engine work
            pending = []

            for c in range(NCH):
                ccols = ds(c * CH, CH)

                # ================= QKV(c): [1344, CH] = W1^T @ x^T =======
                xtt = xt_p.tile([128, ET, CH], FP16, tag="xtt", name="xtt")
                if c == 0:
                    # sliced load spread across four engine queues: right
                    # after kernel start the DMA fabric ramps slowly per
                    # queue, so parallelize the first transfers
                    eng = [nc.gpsimd, nc.scalar, nc.gpsimd, nc.scalar]
                    for s in range(4):
                        eng[s].dma_start(out=xtt[:, ds(4 * s, 4), :],
                                         in_=xt.ap()[c][:, ds(4 * s, 4), :])
                else:
                    nc.sync.dma_start(out=xtt[:], in_=xt.ap()[c])
                qtc = qt_p.tile([128, HPG, CH], FP16, tag="qtc", name="qtc")
                rq = rq_p.tile([128, 2, CH], FP16, tag="rq", name="rq")
                ckv = ckv_p.tile([128, RK // 128, CH], FP16, tag="ckv", name="ckv")

                for di, (kind, idx) in enumerate(dtiles):
                    w1s = w1_p.tile([128, ET, 128], FP16, tag="w1s", name="w1s")
                    nc.sync.dma_start(out=w1s[:], in_=w1.ap()[di])
                    dw = RD if kind == "rk" else 128
                    ps = rot()
                    for e in range(ET):
                        nc.tensor.matmul(ps[:dw, :CH], w1s[:, e, :dw], xtt[:, e, :],
                                         start=(e == 0), stop=(e == ET - 1))
                    if kind == "q":
                        nc.scalar.copy(out=qtc[:, idx, :], in_=ps[:, :CH])
                    elif kind == "ckv":
                        nc.vector.tensor_copy(ckv[:, idx, :], ps[:, :CH])
                    elif kind == "rq":
                        nc.vector.tensor_copy(rq[:, idx, :], ps[:, :CH])
                    else:  # pre-rope k_rope at partitions 0:64
                        nc.vector.tensor_copy(rkd[0:RD, ccols], ps[:RD, :CH])

                if c == 0:
                    load_residents()
                for fn in pending:
                    fn()
                pending = []

                # ================= RoPE(c) ===============================
                # roped = R * cos + (J @ R) * sin   (pairs along partitions)
                for i in range(2):  # q_rope, two head-pair tiles
                    swp = rot()
                    nc.tensor.matmul(swp[:, :CH], jt_t[:, :], rq[:, i, :],
                                     start=True, stop=True)
                    t1 = tmp_p.tile([128, CH], FP16, tag="ropet", name="ropet")
                    nc.vector.tensor_mul(t1[:], rq[:, i, :], cos_sb[:, ccols])
                    nc.vector.tensor_mul(rq[:, i, :], swp[:, :CH], sin_sb[:, ccols])
                    nc.vector.tensor_add(rq[:, i, :], rq[:, i, :], t1[:])
                swp = rot()
                nc.tensor.matmul(swp[:RD, :CH], jt_t[:RD, :RD], rkd[0:RD, ccols],
                                 start=True, stop=True)
                t1 = tmp_p.tile([128, CH], FP16, tag="ropet", name="ropet")
                nc.vector.tensor_mul(t1[:RD, :], rkd[0:RD, ccols], cos_sb[0:RD, ccols])
                nc.vector.tensor_mul(rkd[0:RD, ccols], swp[:RD, :CH], sin_sb[0:RD, ccols])
                nc.vector.tensor_add(rkd[0:RD, ccols], rkd[0:RD, ccols], t1[:RD, :])
                # duplicate roped k_rope to partitions 64:128 (for odd heads)
                nc.sync.dma_start(out=rkd[RD:128, ccols], in_=rkd[0:RD, ccols])

                # ================= UP-K(c): K^T = Wuk^T @ c_kv^T =========
                for h in range(HPG):
                    ps = rot()
                    for kt in range(RK // 128):
                        nc.tensor.matmul(ps[:, :CH], wukt[:, kt, ds(128 * h, 128)],
                                         ckv[:, kt, :],
                                         start=(kt == 0), stop=(kt == RK // 128 - 1))
                    nc.scalar.copy(out=ktc[:, h, ccols], in_=ps[:, :CH])

                # ================= UP-V(c): V = c_kv @ Wuv (L-major) =====
                for lti in range(4):
                    lt = 4 * c + lti
                    for nb in range(2):
                        psv = rot()
                        for kt in range(RK // 128):
                            nc.tensor.matmul(psv[:, :384],
                                             ckv[:, kt, ds(128 * lti, 128)],
                                             wuvt[:, kt, ds(384 * nb, 384)],
                                             start=(kt == 0), stop=(kt == RK // 128 - 1))
                        for q in range(2):
                            hh = 2 * nb + q
                            nc.vector.tensor_copy(vd[:, lt, ds((DV + 1) * hh, DV)],
                                                  psv[:, ds(DV * q, DV)])

                # ================= ATT(c): head pairs, 1-tile pipelined ==
                ntk = 4 * c + 4
                oz = oz_p.tile([128, 6, CH], FP16, tag="oz", name="oz")

                def attn_half(hp, inject=None):
                    heads = (2 * hp, 2 * hp + 1)
                    A = [(acc_p.tile([128, 512], F32, tag="acc1", name="acc1"),
                          acc_p.tile([128, 512], F32, tag="acc2", name="acc2"))
                         for _ in range(2)]
                    geom = []
                    for t in range(ntk):
                        j = t - 4 * c
                        off = 128 * j if j >= 0 else 0
                        geom.append((off, CH - off, j >= 0))
                    sps_l = {}
                    pt_l = {}

                    def emit_qk(t):
                        off, n, _ = geom[t]
                        sps_l[t] = []
                        for q in range(2):
                            h = heads[q]
                            hb = RD * (h % 2)
                            sps = rot()
                            nc.tensor.matmul(sps[:, ds(off, n)],
                                             ktc[:, h, ds(128 * t, 128)],
                                             qtc[:, h, ds(off, n)],
                                             start=True, stop=False)
                            nc.tensor.matmul(sps[:, ds(off, n)],
                                             rkd[hb:hb + RD, ds(128 * t, 128)],
                                             rq[hb:hb + RD, h // 2, ds(off, n)],
                                             start=False, stop=True)
                            sps_l[t].append(sps)

                    def emit_exp(t):
                        off, n, diag = geom[t]
                        pt_l[t] = []
                        for q in range(2):
                            pt = pt_p.tile([128, CH], FP16, tag="pt", name="pt")
                            nc.scalar.activation(pt[:, ds(off, n)],
                                                 sps_l[t][q][:, ds(off, n)],
                                                 AF.Exp, scale=SCALE)
                            if diag:
                                # on Pool: keeps the mask off the DVE, whose
                                # queue carries the slow reciprocals
                                nc.gpsimd.tensor_mul(pt[:, ds(off, 128)],
                                                     pt[:, ds(off, 128)], tri_t[:])
                            pt_l[t].append(pt)

                    def emit_pv(t):
                        off, n, _ = geom[t]
                        for q in range(2):
                            h = heads[q]
                            ps1, ps2 = A[q]
                            pt = pt_l[t][q]
                            nc.tensor.matmul(ps1[:, ds(off, n)],
                                             vd[:, t, ds((DV + 1) * h, 128)],
                                             pt[:, ds(off, n)],
                                             start=(t == 0), stop=(t == ntk - 1),
                                             skip_group_check=True)
                            nc.tensor.matmul(ps2[:DV - DH + 1, ds(off, n)],
                                             vd[:, t, ds((DV + 1) * h + DH, DV - DH + 1)],
                                             pt[:, ds(off, n)],
                                             start=(t == 0), stop=(t == ntk - 1),
                                             skip_group_check=True)

                    # 2-tile software pipeline: PV(t) trails QK(t+2) so the
                    # exp (and diag mask) latency is fully covered by PE work
                    emit_qk(0)
                    emit_qk(1)
                    emit_exp(0)
                    for t in range(2, ntk):
                        emit_qk(t)
                        emit_pv(t - 2)
                        emit_exp(t - 1)
                        if inject is not None and t == min(3, ntk - 1):
                            inject()
                            inject = None
                    emit_pv(ntk - 2)
                    emit_exp(ntk - 1)
                    emit_pv(ntk - 1)
                    if inject is not None:
                        inject()

                    # softmax denominator reciprocal as exp(-ln(d)) on the
                    # Act engine: ~3x faster than the DVE InstReciprocal and
                    # keeps the DVE free for evacuations
                    rrs = []
                    for q in range(2):
                        lnd = tmp_p.tile([1, CH], F32, tag="lnd", name="lnd",
                                         bufs=4)
                        nc.scalar.activation(lnd[:], A[q][1][RD:RD + 1, :CH],
                                             AF.Ln)
                        rr16 = tmp_p.tile([1, CH], FP16, tag="rr16", name="rr16",
                                          bufs=4)
                        nc.scalar.activation(rr16[:], lnd[:], AF.Exp, scale=-1.0)
                        rrs.append(rr16)

                    def finish_norm():
                        for q in range(2):
                            h = heads[q]
                            ps1, ps2 = A[q]
                            rb = rot()
                            nc.tensor.matmul(rb[:, :CH], one_t[:, :], rrs[q][:, :],
                                             start=True, stop=True)
                            rbs = tmp_p.tile([128, CH], FP16, tag="rbs", name="rbs")
                            nc.vector.tensor_copy(rbs[:], rb[:, :CH])
                            nc.vector.tensor_mul(oz[:, h, :], ps1[:, :CH], rbs[:])
                            hb = RD * (h % 2)
                            nc.vector.tensor_mul(oz[hb:hb + RD, 4 + hp, :],
                                                 ps2[0:RD, :CH], rbs[0:RD, :])
                    return finish_norm

                fin0 = attn_half(0)
                fin1 = attn_half(1, inject=fin0)
                pending.append(fin1)

                # ===== FINAL(c): out = attn @ WO, deferred past QKV(c+1) =
                def make_final(c, oz):
                    def emit_final():
                        # keep Pool free during attention: its queue runs the
                        # causal masks, which sit on the exp->PV critical path
                        dma_eng = [nc.sync, nc.scalar, nc.sync, nc.scalar]
                        for eg in range(E // CH):
                            for ls in range(CH // 128):
                                fps = rot()
                                for kt in range(6):
                                    nc.tensor.matmul(fps[:, :CH],
                                                     oz[:, kt, ds(128 * ls, 128)],
                                                     wo_t[:, kt, ds(CH * eg, CH)],
                                                     start=(kt == 0), stop=(kt == 5))
                                fin = fin_p.tile([128, CH], FP16, tag="fin",
                                                 name="fin")
                                # alternate evac + store queues so the last
                                # chunk's drain doesn't serialize on one engine
                                if ls % 2 == 0:
                                    nc.scalar.copy(out=fin[:], in_=fps[:, :CH])
                                else:
                                    nc.vector.tensor_copy(fin[:], fps[:, :CH])
                                dma_eng[ls].dma_start(
                                    out=outt.ap()[ds(c * CH + 128 * ls, 128),
                                                  ds(CH * eg, CH)],
                                    in_=fin[:])
                    return emit_final

                pending.append(make_final(c, oz))

            for fn in pending:
                fn()
            pending = []

    _split_excess_waits(nc)
    return nc


def _prep_inputs(x, cos_table, sin_table, wq, wkv_down, w_up, w_out):
    f32 = np.float32
    wq3 = np.asarray(wq, f32).reshape(E, H, DV)
    wup3 = np.asarray(w_up, f32).reshape(RK, H, 2 * DH + RD)
    wo3 = np.asarray(w_out, f32).reshape(H, DV, E)
    wkv = np.asarray(wkv_down, f32)

    cosI = np.repeat(np.asarray(cos_table, f32)[:L], 2, axis=1).T  # [64, L]
    sinI = np.repeat(np.asarray(sin_table, f32)[:L], 2, axis=1).T
    cost = np.ascontiguousarray(np.concatenate([cosI, cosI], 0)).astype(NPFP16)
    sint = np.ascontiguousarray(np.concatenate([sinI, sinI], 0)).astype(NPFP16)
    J = np.zeros((128, 128), f32)
    for i in range(64):
        J[2 * i, 2 * i + 1] = -1.0
        J[2 * i + 1, 2 * i] = 1.0
    jt = np.ascontiguousarray(J.T).astype(NPFP16)
    triu = np.ascontiguousarray(np.triu(np.ones((128, 128), f32))).astype(NPFP16)

    in_maps = []
    for core in range(NCORE):
        b, g = core // HPG, core % HPG
        hs = slice(HPG * g, HPG * g + HPG)
        xT = np.asarray(x, f32)[b].T                       # [E, L]
        xt_pack = np.ascontiguousarray(
            xT.reshape(ET, 128, NCH, CH).transpose(2, 1, 0, 3)).astype(NPFP16)
        wq_c = wq3[:, hs, :DH].reshape(E, HPG * DH)
        wq_r = wq3[:, hs, DH:].reshape(E, HPG * RD)
        w1_flat = np.concatenate([wq_c, wkv[:, :RK], wq_r, wkv[:, RK:]], axis=1)
        # pack into 11 d-strips [128, ET, 128] (last strip: 64 cols, zero-pad)
        w1_pack = np.zeros((11, 128, ET, 128), f32)
        offs = [128 * i for i in range(10)] + [1280]
        wids = [128] * 10 + [64]
        for di, (o, w) in enumerate(zip(offs, wids)):
            w1_pack[di, :, :, :w] = (
                w1_flat[:, o:o + w].reshape(ET, 128, w).transpose(1, 0, 2))
        # w_out resident: 4 content strips + 2 rope-pair strips
        wo_pack = np.zeros((128, 6, E), f32)
        wog = wo3[hs]                                      # [4, 192, E]
        for kt in range(HPG):
            wo_pack[:, kt, :] = wog[kt, :DH, :]
        for hp in range(2):
            wo_pack[0:RD, 4 + hp, :] = wog[2 * hp, DH:, :]
            wo_pack[RD:128, 4 + hp, :] = wog[2 * hp + 1, DH:, :]
        in_maps.append({
            "xt": xt_pack,
            "w1": w1_pack.astype(NPFP16),
            "wuk": np.ascontiguousarray(
                wup3[:, hs, :DH].reshape(RK, HPG * DH)).astype(NPFP16),
            "wuv": np.ascontiguousarray(
                wup3[:, hs, DH:].reshape(RK, HPG * DV)).astype(NPFP16),
            "wo": wo_pack.astype(NPFP16),
            "cost": cost,
            "sint": sint,
            "jt": jt,
            "triu": triu,
            "ones1": np.ones((1, 128), NPFP16),
        })
    return in_maps


def kernel(x, cos_table, sin_table, wq, wkv_down, w_up, w_out, _want_perf=False):
    if "nc" not in _CACHE:
        _CACHE["nc"] = _build()
    nc = _CACHE["nc"]
    in_maps = _prep_inputs(x, cos_table, sin_table, wq, wkv_down, w_up, w_out)
    res = run_bass_kernel_spmd(nc, in_maps, core_ids=list(range(NCORE)),
                               trace=bool(_want_perf),
                               tmpdir=os.environ.get("BASS_TMPDIR") or None)
    out = np.zeros((B, L, E), np.float32)
    for core in range(NCORE):
        b = core // HPG
        out[b] += res.results[core]["outt"].astype(np.float32)
    if _want_perf:
        return out, res
    return out
